# revision 1
# baseline (speedup 1.0000x reference)
"""MiniBatchDiscrimination Trainium2 kernel.

reference:
    M = einsum('nhwf,fbc->nhwbc', x, T)          # [N,H,W,B,C]
    norm = sum_c |M[i] - M[j]|                   # [N,N,H,W,B]
    o_b  = sum_j exp(-norm)                      # [N,H,W,B]
    out  = concat([x, o_b], axis=3)              # [N,H,W,F+B]

Sharding: data-parallel over the outer batch axis N (4 rows per core, 8
cores); every core receives the full x (as a pre-transposed fp16 copy) and
computes the whole M on-chip, then only its 4 rows of the pairwise block.

Per-core device layout ("L2"): M2_q [(b16,c8) partitions, (n32,hw256) free]
for each b-quarter q, so that
  - M-compute is a plain matmul (lhsT = T-tile [f,(b,c)], rhs = xT [f,(n,hw)])
  - the pairwise |M_j - M_i| is a DVE tensor op between free-dim slices
    (j-block read vs broadcast i-slice)
  - the c-reduction contracts the partition axis on the TensorEngine with
    stripe-ones matrices, accumulating 8 (q',i) stripes into one PSUM tile
    [(q',i,b) partitions, (j,hw) free]
  - exp(-norm) is one ACT pass, and the j-sum is a windowed DVE reduce
    over the strided j axis.
"""

import os
import sys

for _p in ("/opt/trn_rl_repo", "/opt/pypackages"):
    if _p not in sys.path and os.path.isdir(_p):
        sys.path.append(_p)

import numpy as np

N, HW, F, B, C = 32, 256, 256, 64, 8
NL = 4          # local rows per core
CORES = 8
FH = 2          # f in two partition halves of 128
Q = 4           # b-quarters of 16
HWC = 4         # hw chunks of 64
HW_CH = HW // HWC

F16 = "float16"

# bisect knobs (set before _get_program): ABS_MODE in {"custom", "stock"};
# STAGES subset of {"B", "C"} to build
ABS_MODE = "custom"
PERF_MAX = 1
STAGES = {"B", "C"}


def _absdiff_uop_1x():
    """REGULAR program: |a-b| via SUB, reverse-SUB, MAX on slices 0-2."""
    from concourse.dve_uop import (
        ENABLE, AluInp, AluOp, DelayInp, InpSel, OutPath, OutSel, Trigger,
        UopConfig, UopDpConfig,
    )

    u = UopConfig()
    u.enable_input(InpSel.SRC_0, 0).enable_input(InpSel.SRC_1, 1)
    u.require_inp0 = ENABLE
    u.require_inp1 = ENABLE
    u.trigger = (Trigger.SRC_TENSOR_DONE, Trigger.NONE, Trigger.NONE)
    u.enable_output(OutSel.ALU_OUT, OutPath.WR0_LO)
    dp = u.datapath_config
    # s0: alu = a - b; carry b (chain0), capture a (chain3)
    dp[0] = (UopDpConfig()
             .enable_alu(AluOp.SUBTRACT, AluInp.PREV_ALU_OUT, AluInp.PREV_DELAY_0)
             .pass_through_delay(0)
             .enable_delay_from_src(DelayInp.PREV_ALU_OUT, 3))
    # s1: alu = b - a; capture (a-b) into chain0
    dp[1] = (UopDpConfig()
             .enable_alu(AluOp.SUBTRACT, AluInp.PREV_DELAY_0, AluInp.PREV_DELAY_3)
             .enable_delay_from_src(DelayInp.PREV_ALU_OUT, 0))
    # s2: alu = max(b-a, a-b)
    dp[2] = UopDpConfig().enable_alu(
        AluOp.MAX, AluInp.PREV_ALU_OUT, AluInp.PREV_DELAY_0)
    for i in range(3, 8):
        dp[i] = UopDpConfig().pass_through_alu()
    return u


def _absdiff_uop_2x():
    """2X_1PORT program: lo on slices 0-2, hi on slices 3-5."""
    from concourse.dve_uop import (
        ENABLE, AluInp, AluOp, DelayInp, InpSel, OutPath, OutSel, Trigger,
        UopConfig, UopDpConfig,
    )

    u = UopConfig()
    u.enable_input(InpSel.SRC_0, 0).enable_input(InpSel.SRC_1, 1)
    u.enable_input(InpSel.SRC_0_HI, 2).enable_input(InpSel.SRC_1_HI, 3)
    u.require_inp0 = ENABLE
    u.require_inp1 = ENABLE
    u.trigger = (Trigger.SRC_TENSOR_DONE, Trigger.NONE, Trigger.NONE)
    u.enable_output(OutSel.DELAY_0, OutPath.WR0_LO)   # lo result rides chain0
    u.enable_output(OutSel.ALU_OUT, OutPath.WR0_HI)   # hi result on ALU lane
    dp = u.datapath_config
    # s0: alu = a_lo - b_lo; carry b_lo(c0), a_hi(c1), b_hi(c2); capture a_lo(c3)
    dp[0] = (UopDpConfig()
             .enable_alu(AluOp.SUBTRACT, AluInp.PREV_ALU_OUT, AluInp.PREV_DELAY_0)
             .pass_through_delay(0, 1, 2)
             .enable_delay_from_src(DelayInp.PREV_ALU_OUT, 3))
    # s1: alu = b_lo - a_lo; capture (a-b)_lo into c0; carry a_hi, b_hi
    dp[1] = (UopDpConfig()
             .enable_alu(AluOp.SUBTRACT, AluInp.PREV_DELAY_0, AluInp.PREV_DELAY_3)
             .enable_delay_from_src(DelayInp.PREV_ALU_OUT, 0)
             .pass_through_delay(1, 2))
    # s2: alu = max -> |a-b|_lo; carry a_hi, b_hi
    dp[2] = (UopDpConfig()
             .enable_alu(AluOp.MAX, AluInp.PREV_ALU_OUT, AluInp.PREV_DELAY_0)
             .pass_through_delay(1, 2))
    # s3: alu = a_hi - b_hi; capture lo result into c0; carry a_hi, b_hi
    dp[3] = (UopDpConfig()
             .enable_alu(AluOp.SUBTRACT, AluInp.PREV_DELAY_1, AluInp.PREV_DELAY_2)
             .enable_delay_from_src(DelayInp.PREV_ALU_OUT, 0)
             .pass_through_delay(1, 2))
    # s4: alu = b_hi - a_hi; carry lo(c0); capture (a-b)_hi into c3
    dp[4] = (UopDpConfig()
             .enable_alu(AluOp.SUBTRACT, AluInp.PREV_DELAY_2, AluInp.PREV_DELAY_1)
             .pass_through_delay(0)
             .enable_delay_from_src(DelayInp.PREV_ALU_OUT, 3))
    # s5: alu = max -> |a-b|_hi; carry lo(c0)
    dp[5] = (UopDpConfig()
             .enable_alu(AluOp.MAX, AluInp.PREV_ALU_OUT, AluInp.PREV_DELAY_3)
             .pass_through_delay(0))
    # s6, s7: pass alu (hi) + chain0 (lo)
    for i in (6, 7):
        dp[i] = UopDpConfig().pass_through_alu().pass_through_delay(0)
    return u


def _get_absdiff_op():
    """Fused |a-b| custom DVE op with a hand-written 2X_1PORT variant."""
    if "absdiff" in _CACHED:
        return _CACHED["absdiff"]
    from concourse import dve_ops
    from concourse.dve_spec import Spec, Src0, Src1, maxx
    from concourse.dve_uop import DveOpSpec

    NAME = "ABSDIFF_ANT"
    for op in dve_ops.OPS:
        if op.name == NAME:
            _CACHED["absdiff"] = op
            return op
    spec = Spec(
        body=maxx(Src0 - Src1, Src1 - Src0),
        reference=lambda in0, in1, s0, s1, imm2: np.abs(
            in0.astype(np.float32) - in1.astype(np.float32)
        ),
    )
    op = dve_ops.DveOp(NAME, spec, subdim=False, uops_sha={})
    dve_ops.OPS.append(op)
    dve_ops.CUSTOM_DVE_SPECS[op.name] = op.spec
    row = dve_ops._CUSTOM_DVE_ROW_BASE + len(dve_ops.OPS) - 1
    dve_ops._SUB_OPCODE_FOR_NAME[op.name] = row
    compiled = DveOpSpec(
        name=NAME,
        opcode=row,
        uops=[_absdiff_uop_1x()],
        uops_2x=[_absdiff_uop_2x()],
        perf_max=1,
        rd1_en=True,
    )
    compiled.validate("v3")
    dve_ops._COMPILE_CACHE[(NAME, "v3")] = compiled
    dve_ops._COMPILE_CACHE[(NAME, "v4")] = compiled
    _CACHED["absdiff"] = op
    return op


# --------------------------------------------------------------------------
# device program
# --------------------------------------------------------------------------

def make_pools(tc, ctx, rep=0):
    sfx = f"_{rep}"
    singles = ctx.enter_context(tc.tile_pool(name="singles" + sfx, bufs=1))
    psA = ctx.enter_context(tc.tile_pool(name="psA" + sfx, bufs=2, space="PSUM"))
    psN = ctx.enter_context(tc.tile_pool(name="psN" + sfx, bufs=4, space="PSUM"))
    adp = ctx.enter_context(tc.tile_pool(name="adp" + sfx, bufs=10))
    Ep = ctx.enter_context(tc.tile_pool(name="Ep" + sfx, bufs=3))
    return singles, psA, psN, adp, Ep


def build_body(tc, outs, ins, rep=0, pools=None):
    """Trace the per-core Tile program.

    ins:  xT   [2,128,8192] f16   xT[fh,f,n*256+hw] = x[n,hw,fh*128+f]
          xiT  [2,128,1024] f16   same, restricted to this core's 4 rows
          tw   [2,4,128,128] f16  tw[fh,q,f,b*8+c] = T[fh*128+f,16q+b,c]
          ones [8,128,128]  f16   ones[s,b*8+c,col] = (col == 16s+b)
    outs: o    [2,128,256]  f32   o[t, 64q'+16i+b, hw] = o_b[ib+i, hw, 16(2t+q')+b]
    """
    from contextlib import ExitStack

    import concourse.bass as bass
    import concourse.mybir as mybir

    nc = tc.nc
    f16 = mybir.dt.float16
    f32 = mybir.dt.float32

    xT_d, xiT_d, tw_d, ones_d = ins["xT"], ins["xiT"], ins["tw"], ins["ones"]
    o_d = outs["o"]

    with ExitStack() as ctx:
        if pools is None:
            pools = make_pools(tc, ctx, rep)
        singles, psA, psN, adp, Ep = pools

        # ---- loads -------------------------------------------------------
        xT_s, xiT_s, tw_s = [], [], []
        for fh in range(FH):
            t = singles.tile([128, N * HW], f16, tag=f"xT{fh}")
            nc.sync.dma_start(out=t, in_=xT_d[fh])
            xT_s.append(t)
            t = singles.tile([128, NL * HW], f16, tag=f"xiT{fh}")
            nc.sync.dma_start(out=t, in_=xiT_d[fh])
            xiT_s.append(t)
            row = []
            for q in range(Q):
                t = singles.tile([128, 128], f16, tag=f"tw{fh}{q}")
                nc.sync.dma_start(out=t, in_=tw_d[fh, q])
                row.append(t)
            tw_s.append(row)
        ones_s = []
        for s in range(8):
            t = singles.tile([128, 128], f16, tag=f"ones{s}")
            nc.sync.dma_start(out=t, in_=ones_d[s])
            ones_s.append(t)

        # ---- stage B (per b-quarter-pair) interleaved with stage C -------
        M2, M2i = [], []
        if "B" not in STAGES:
            for q in range(Q):
                m2 = singles.tile([128, N * HW], f16, tag=f"m2{q}")
                nc.vector.memset(m2[:, 0:16], 0.0)
                M2.append(m2)
                m2i = singles.tile([128, NL * HW], f16, tag=f"m2i{q}")
                nc.vector.memset(m2i[:, 0:16], 0.0)
                M2i.append(m2i)
        def stage_b(q):
            # local-row tile first: stage C's earliest dependency
            m2i = singles.tile([128, NL * HW], f16, tag=f"m2i{q}")
            ps = psA.tile([128, 1024], f32, tag="psA")
            for sub in range(2):
                sl = slice(sub * 512, (sub + 1) * 512)
                for fh in range(FH):
                    nc.tensor.matmul(
                        ps[:, sl], lhsT=tw_s[fh][q], rhs=xiT_s[fh][:, sl],
                        start=(fh == 0), stop=(fh == 1),
                    )
            nc.scalar.copy(out=m2i, in_=ps[:])
            M2i.append(m2i)
            m2 = singles.tile([128, N * HW], f16, tag=f"m2{q}")
            for piece in range(8):          # 1024-col pieces; fh-outer shares LDW
                ps = psA.tile([128, 1024], f32, tag="psA")
                for fh in range(FH):
                    for sub in range(2):    # 512-col matmuls
                        sl = slice(sub * 512, (sub + 1) * 512)
                        src = slice(piece * 1024 + sub * 512,
                                    piece * 1024 + (sub + 1) * 512)
                        nc.tensor.matmul(
                            ps[:, sl], lhsT=tw_s[fh][q], rhs=xT_s[fh][:, src],
                            start=(fh == 0), stop=(fh == 1),
                        )
                nc.scalar.copy(out=m2[:, piece * 1024:(piece + 1) * 1024], in_=ps[:])
            M2.append(m2)

        if "B" in STAGES:
            stage_b(0)
            stage_b(1)

        # ---- stage C: pairwise |diff|, c-reduce, exp, j-sum --------------
        for t in range(2 if "C" in STAGES else 0):
            if "B" in STAGES and t == 1:
                stage_b(2)
                stage_b(3)                  # b-quarter pairs (q = 2t+q')
            o_sb = singles.tile([128, HW], f32, tag=f"osb{t}")
            for hwc in range(HWC):
                hsl = slice(hwc * HW_CH, (hwc + 1) * HW_CH)
                ads = []
                for qp in range(2):
                    q = 2 * t + qp
                    m2v = M2[q].rearrange("p (n hw) -> p n hw", n=N)
                    m2iv = M2i[q].rearrange("p (i hw) -> p i hw", i=NL)
                    for i in range(NL):
                        ad = adp.tile([128, N * HW_CH], f16, tag="ad")
                        adv = ad.rearrange("p (n hw) -> p n hw", n=N)
                        src0 = m2v[:, :, hsl]
                        s1 = m2iv[:, i, hsl]           # [128, 64]
                        src1 = bass.AP(
                            tensor=s1.tensor, offset=s1.offset,
                            ap=[list(s1.ap[0]), [0, N], list(s1.ap[1])],
                        )
                        if ABS_MODE == "subonly":   # timing probe: wrong numerics
                            nc.vector.tensor_sub(adv, src0, src1)
                        elif ABS_MODE == "custom":
                            bi = nc.vector._custom_dve(
                                _get_absdiff_op(), out=adv, in0=src0, in1=src1,
                            )
                            bi.ins.perf_max = PERF_MAX
                        else:
                            nc.vector.tensor_sub(adv, src0, src1)
                            nc.vector.scalar_tensor_tensor(
                                out=ad, in0=ad, scalar=-1.0, in1=ad,
                                op0=mybir.AluOpType.mult, op1=mybir.AluOpType.max,
                            )
                        ads.append(ad)
                E = Ep.tile([128, N * HW_CH], f32, tag="E")
                nrms = []
                for _h in range(4):
                    nrm = psN.tile([128, 512], f32, tag="nrm")
                    nrms.append(nrm)
                for s in range(8):          # s-outer: one weight-load per stripe
                    for h in range(4):
                        nc.tensor.matmul(
                            nrms[h][:], lhsT=ones_s[s],
                            rhs=ads[s][:, h * 512:(h + 1) * 512],
                            start=(s == 0), stop=(s == 7),
                        )
                for h in range(4):
                    nc.scalar.activation(
                        out=E[:, h * 512:(h + 1) * 512], in_=nrms[h][:],
                        func=mybir.ActivationFunctionType.Exp, scale=-1.0,
                    )
                Ev = E.rearrange("p (j hw) -> p hw j", j=N)
                nc.vector.tensor_reduce(
                    out=o_sb[:, hsl], in_=Ev,
                    axis=mybir.AxisListType.X, op=mybir.AluOpType.add,
                )
            nc.sync.dma_start(out=o_d[t], in_=o_sb)


# --------------------------------------------------------------------------
# host side
# --------------------------------------------------------------------------

def prep_inputs(x, T):
    """Shared (core-independent) device inputs."""
    xf = np.ascontiguousarray(x.reshape(N, HW, F))
    xT_np = np.ascontiguousarray(xf.transpose(2, 0, 1).reshape(F, N * HW))
    xT_in = xT_np.reshape(FH, 128, N * HW).astype(np.float16)
    tw = T.reshape(FH, 128, Q, 16, C).transpose(0, 2, 1, 3, 4)
    tw_in = np.ascontiguousarray(tw.reshape(FH, Q, 128, 128)).astype(np.float16)
    ones_in = np.zeros((8, 128, 128), np.float16)
    for s in range(8):
        for b in range(16):
            ones_in[s, b * 8:(b + 1) * 8, 16 * s + b] = 1.0
    return xT_np, xT_in, tw_in, ones_in


def core_in_map(xT_np, xT_in, tw_in, ones_in, k):
    xiT = np.ascontiguousarray(
        xT_np[:, k * NL * HW:(k + 1) * NL * HW]
    ).reshape(FH, 128, NL * HW).astype(np.float16)
    return {"xT": xT_in, "xiT": xiT, "tw": tw_in, "ones": ones_in}


def gather_ob(core_outs):
    """core_outs: list of 8 arrays [2,128,256] f32 -> o_b [N,16,16,B]."""
    obs = []
    for res in core_outs:
        v = res.astype(np.float32).reshape(2, 2, NL, 16, HW)   # t, q', i, b, hw
        obs.append(v.transpose(2, 4, 0, 1, 3).reshape(NL, HW, B))
    return np.concatenate(obs, axis=0).reshape(N, 16, 16, B)


_CACHED = {}


def _get_program(reps=1, loop=None):
    key = ("nc", reps, loop)
    if key in _CACHED:
        return _CACHED[key]
    from contextlib import ExitStack
    import concourse.bacc as bacc
    import concourse.mybir as mybir
    import concourse.tile as tile

    nc = bacc.Bacc("TRN2", target_bir_lowering=False, debug=False,
                   num_devices=CORES)
    f16, f32 = mybir.dt.float16, mybir.dt.float32
    ins = {
        "xT": nc.dram_tensor("xT", [FH, 128, N * HW], f16, kind="ExternalInput").ap(),
        "xiT": nc.dram_tensor("xiT", [FH, 128, NL * HW], f16, kind="ExternalInput").ap(),
        "tw": nc.dram_tensor("tw", [FH, Q, 128, 128], f16, kind="ExternalInput").ap(),
        "ones": nc.dram_tensor("ones", [8, 128, 128], f16, kind="ExternalInput").ap(),
    }
    outs = {
        "o": nc.dram_tensor("o", [2, 128, HW], f32, kind="ExternalOutput").ap(),
    }
    with tile.TileContext(nc) as tc:
        if loop:
            with ExitStack() as ctx:
                pools = make_pools(tc, ctx)
                with tc.For_i(0, loop, 1,
                              hint_engines=(mybir.EngineType.PE,
                                            mybir.EngineType.DVE)):
                    build_body(tc, outs, ins, pools=pools)
        else:
            for r in range(reps):
                build_body(tc, outs, ins, rep=r)
    nc.compile()
    _CACHED[key] = nc
    return nc


def kernel(x, T):
    x = np.asarray(x, dtype=np.float32)
    T = np.asarray(T, dtype=np.float32)
    from concourse.bass_utils import run_bass_kernel_spmd

    nc = _get_program()
    xT_np, xT_in, tw_in, ones_in = prep_inputs(x, T)
    in_maps = [core_in_map(xT_np, xT_in, tw_in, ones_in, k) for k in range(CORES)]
    res = run_bass_kernel_spmd(nc, in_maps, core_ids=list(range(CORES)))
    ob = gather_ob([r["o"] for r in res.results])
    return np.concatenate([x, ob], axis=3)



# revision 6
# speedup vs baseline: 1.0120x; 1.0120x over previous
"""MiniBatchDiscrimination Trainium2 kernel.

reference:
    M = einsum('nhwf,fbc->nhwbc', x, T)          # [N,H,W,B,C]
    norm = sum_c |M[i] - M[j]|                   # [N,N,H,W,B]
    o_b  = sum_j exp(-norm)                      # [N,H,W,B]
    out  = concat([x, o_b], axis=3)              # [N,H,W,F+B]

Sharding: data-parallel over the outer batch axis N (4 rows per core, 8
cores); every core receives the full x (as a pre-transposed fp16 copy) and
computes the whole M on-chip, then only its 4 rows of the pairwise block.

Per-core device layout ("L2"): M2_q [(b16,c8) partitions, (n32,hw256) free]
for each b-quarter q, so that
  - M-compute is a plain matmul (lhsT = T-tile [f,(b,c)], rhs = xT [f,(n,hw)])
  - the pairwise |M_j - M_i| is a DVE tensor op between free-dim slices
    (j-block read vs broadcast i-slice)
  - the c-reduction contracts the partition axis on the TensorEngine with
    stripe-ones matrices, accumulating 8 (q',i) stripes into one PSUM tile
    [(q',i,b) partitions, (j,hw) free]
  - exp(-norm) is one ACT pass, and the j-sum is a windowed DVE reduce
    over the strided j axis.
"""

import os
import sys

for _p in ("/opt/trn_rl_repo", "/opt/pypackages"):
    if _p not in sys.path and os.path.isdir(_p):
        sys.path.append(_p)

import numpy as np

N, HW, F, B, C = 32, 256, 256, 64, 8
NL = 4          # local rows per core
CORES = 8
FH = 2          # f in two partition halves of 128
Q = 4           # b-quarters of 16
HWC = 4         # hw chunks of 64
HW_CH = HW // HWC

F16 = "float16"

# bisect knobs (set before _get_program): ABS_MODE in {"custom", "stock"};
# STAGES subset of {"B", "C"} to build
ABS_MODE = "custom"
PERF_MAX = 1
STAGES = {"B", "C"}


def _absdiff_uop_1x():
    """REGULAR program: |a-b| via SUB, reverse-SUB, MAX on slices 0-2."""
    from concourse.dve_uop import (
        ENABLE, AluInp, AluOp, DelayInp, InpSel, OutPath, OutSel, Trigger,
        UopConfig, UopDpConfig,
    )

    u = UopConfig()
    u.enable_input(InpSel.SRC_0, 0).enable_input(InpSel.SRC_1, 1)
    u.require_inp0 = ENABLE
    u.require_inp1 = ENABLE
    u.trigger = (Trigger.SRC_TENSOR_DONE, Trigger.NONE, Trigger.NONE)
    u.enable_output(OutSel.ALU_OUT, OutPath.WR0_LO)
    dp = u.datapath_config
    # s0: alu = a - b; carry b (chain0), capture a (chain3)
    dp[0] = (UopDpConfig()
             .enable_alu(AluOp.SUBTRACT, AluInp.PREV_ALU_OUT, AluInp.PREV_DELAY_0)
             .pass_through_delay(0)
             .enable_delay_from_src(DelayInp.PREV_ALU_OUT, 3))
    # s1: alu = b - a; capture (a-b) into chain0
    dp[1] = (UopDpConfig()
             .enable_alu(AluOp.SUBTRACT, AluInp.PREV_DELAY_0, AluInp.PREV_DELAY_3)
             .enable_delay_from_src(DelayInp.PREV_ALU_OUT, 0))
    # s2: alu = max(b-a, a-b)
    dp[2] = UopDpConfig().enable_alu(
        AluOp.MAX, AluInp.PREV_ALU_OUT, AluInp.PREV_DELAY_0)
    for i in range(3, 8):
        dp[i] = UopDpConfig().pass_through_alu()
    return u


def _absdiff_uop_2x():
    """2X_1PORT program: lo on slices 0-2, hi on slices 3-5."""
    from concourse.dve_uop import (
        ENABLE, AluInp, AluOp, DelayInp, InpSel, OutPath, OutSel, Trigger,
        UopConfig, UopDpConfig,
    )

    u = UopConfig()
    u.enable_input(InpSel.SRC_0, 0).enable_input(InpSel.SRC_1, 1)
    u.enable_input(InpSel.SRC_0_HI, 2).enable_input(InpSel.SRC_1_HI, 3)
    u.require_inp0 = ENABLE
    u.require_inp1 = ENABLE
    u.trigger = (Trigger.SRC_TENSOR_DONE, Trigger.NONE, Trigger.NONE)
    u.enable_output(OutSel.DELAY_0, OutPath.WR0_LO)   # lo result rides chain0
    u.enable_output(OutSel.ALU_OUT, OutPath.WR0_HI)   # hi result on ALU lane
    dp = u.datapath_config
    # s0: alu = a_lo - b_lo; carry b_lo(c0), a_hi(c1), b_hi(c2); capture a_lo(c3)
    dp[0] = (UopDpConfig()
             .enable_alu(AluOp.SUBTRACT, AluInp.PREV_ALU_OUT, AluInp.PREV_DELAY_0)
             .pass_through_delay(0, 1, 2)
             .enable_delay_from_src(DelayInp.PREV_ALU_OUT, 3))
    # s1: alu = b_lo - a_lo; capture (a-b)_lo into c0; carry a_hi, b_hi
    dp[1] = (UopDpConfig()
             .enable_alu(AluOp.SUBTRACT, AluInp.PREV_DELAY_0, AluInp.PREV_DELAY_3)
             .enable_delay_from_src(DelayInp.PREV_ALU_OUT, 0)
             .pass_through_delay(1, 2))
    # s2: alu = max -> |a-b|_lo; carry a_hi, b_hi
    dp[2] = (UopDpConfig()
             .enable_alu(AluOp.MAX, AluInp.PREV_ALU_OUT, AluInp.PREV_DELAY_0)
             .pass_through_delay(1, 2))
    # s3: alu = a_hi - b_hi; capture lo result into c0; carry a_hi, b_hi
    dp[3] = (UopDpConfig()
             .enable_alu(AluOp.SUBTRACT, AluInp.PREV_DELAY_1, AluInp.PREV_DELAY_2)
             .enable_delay_from_src(DelayInp.PREV_ALU_OUT, 0)
             .pass_through_delay(1, 2))
    # s4: alu = b_hi - a_hi; carry lo(c0); capture (a-b)_hi into c3
    dp[4] = (UopDpConfig()
             .enable_alu(AluOp.SUBTRACT, AluInp.PREV_DELAY_2, AluInp.PREV_DELAY_1)
             .pass_through_delay(0)
             .enable_delay_from_src(DelayInp.PREV_ALU_OUT, 3))
    # s5: alu = max -> |a-b|_hi; carry lo(c0)
    dp[5] = (UopDpConfig()
             .enable_alu(AluOp.MAX, AluInp.PREV_ALU_OUT, AluInp.PREV_DELAY_3)
             .pass_through_delay(0))
    # s6, s7: pass alu (hi) + chain0 (lo)
    for i in (6, 7):
        dp[i] = UopDpConfig().pass_through_alu().pass_through_delay(0)
    return u


def _get_absdiff_op():
    """Fused |a-b| custom DVE op with a hand-written 2X_1PORT variant."""
    if "absdiff" in _CACHED:
        return _CACHED["absdiff"]
    from concourse import dve_ops
    from concourse.dve_spec import Spec, Src0, Src1, maxx
    from concourse.dve_uop import DveOpSpec

    NAME = "ABSDIFF_ANT"
    for op in dve_ops.OPS:
        if op.name == NAME:
            _CACHED["absdiff"] = op
            return op
    spec = Spec(
        body=maxx(Src0 - Src1, Src1 - Src0),
        reference=lambda in0, in1, s0, s1, imm2: np.abs(
            in0.astype(np.float32) - in1.astype(np.float32)
        ),
    )
    op = dve_ops.DveOp(NAME, spec, subdim=False, uops_sha={})
    dve_ops.OPS.append(op)
    dve_ops.CUSTOM_DVE_SPECS[op.name] = op.spec
    row = dve_ops._CUSTOM_DVE_ROW_BASE + len(dve_ops.OPS) - 1
    dve_ops._SUB_OPCODE_FOR_NAME[op.name] = row
    compiled = DveOpSpec(
        name=NAME,
        opcode=row,
        uops=[_absdiff_uop_1x()],
        uops_2x=[_absdiff_uop_2x()],
        perf_max=1,
        rd1_en=True,
    )
    compiled.validate("v3")
    dve_ops._COMPILE_CACHE[(NAME, "v3")] = compiled
    dve_ops._COMPILE_CACHE[(NAME, "v4")] = compiled
    _CACHED["absdiff"] = op
    return op


# --------------------------------------------------------------------------
# device program
# --------------------------------------------------------------------------

def make_pools(tc, ctx, rep=0):
    sfx = f"_{rep}"
    singles = ctx.enter_context(tc.tile_pool(name="singles" + sfx, bufs=1))
    psA = ctx.enter_context(tc.tile_pool(name="psA" + sfx, bufs=2, space="PSUM"))
    psN = ctx.enter_context(tc.tile_pool(name="psN" + sfx, bufs=4, space="PSUM"))
    adp = ctx.enter_context(tc.tile_pool(name="adp" + sfx, bufs=10))
    Ep = ctx.enter_context(tc.tile_pool(name="Ep" + sfx, bufs=3))
    return singles, psA, psN, adp, Ep


def build_body(tc, outs, ins, rep=0, pools=None):
    """Trace the per-core Tile program.

    ins:  xT   [2,128,8192] f16   xT[fh,f,n*256+hw] = x[n,hw,fh*128+f]
          xiT  [2,128,1024] f16   same, restricted to this core's 4 rows
          tw   [2,4,128,128] f16  tw[fh,q,f,b*8+c] = T[fh*128+f,16q+b,c]
          ones [8,128,32]   f16   ones[s,b*8+c,col] = (col == 16*(s%2)+b)
                                  (col-tiled: stripe s -> col group s//2)
    outs: o    [2,128,256]  f32   o[t, 64q'+16i+b, hw] = o_b[ib+i, hw, 16(2t+q')+b]
    """
    from contextlib import ExitStack

    import concourse.bass as bass
    import concourse.mybir as mybir

    nc = tc.nc
    f16 = mybir.dt.float16
    f32 = mybir.dt.float32

    xT_d, xiT_d, tw_d, ones_d = ins["xT"], ins["xiT"], ins["tw"], ins["ones"]
    o_d = outs["o"]

    with ExitStack() as ctx:
        if pools is None:
            pools = make_pools(tc, ctx, rep)
        singles, psA, psN, adp, Ep = pools

        # ---- loads -------------------------------------------------------
        xT_s, xiT_s, tw_s = [], [], []
        for fh in range(FH):
            t = singles.tile([128, N * HW], f16, tag=f"xT{fh}")
            nc.sync.dma_start(out=t, in_=xT_d[fh])
            xT_s.append(t)
            t = singles.tile([128, NL * HW], f16, tag=f"xiT{fh}")
            nc.sync.dma_start(out=t, in_=xiT_d[fh])
            xiT_s.append(t)
            row = []
            for q in range(Q):
                t = singles.tile([128, 128], f16, tag=f"tw{fh}{q}")
                nc.sync.dma_start(out=t, in_=tw_d[fh, q])
                row.append(t)
            tw_s.append(row)
        ones_s = []
        for s in range(8):
            t = singles.tile([128, 32], f16, tag=f"ones{s}")
            nc.sync.dma_start(out=t, in_=ones_d[s])
            ones_s.append(t)

        # ---- stage B (per b-quarter-pair) interleaved with stage C -------
        M2, M2i = [], []
        if "B" not in STAGES:
            for q in range(Q):
                m2 = singles.tile([128, N * HW], f16, tag=f"m2{q}")
                nc.vector.memset(m2[:, 0:16], 0.0)
                M2.append(m2)
                m2i = singles.tile([128, NL * HW], f16, tag=f"m2i{q}")
                nc.vector.memset(m2i[:, 0:16], 0.0)
                M2i.append(m2i)
        def stage_b(q):
            # local-row tile first: stage C's earliest dependency
            m2i = singles.tile([128, NL * HW], f16, tag=f"m2i{q}")
            ps = psA.tile([128, 1024], f32, tag="psA")
            for sub in range(2):
                sl = slice(sub * 512, (sub + 1) * 512)
                for fh in range(FH):
                    nc.tensor.matmul(
                        ps[:, sl], lhsT=tw_s[fh][q], rhs=xiT_s[fh][:, sl],
                        start=(fh == 0), stop=(fh == 1),
                    )
            nc.scalar.copy(out=m2i, in_=ps[:])
            M2i.append(m2i)
            m2 = singles.tile([128, N * HW], f16, tag=f"m2{q}")
            for piece in range(8):          # 1024-col pieces; fh-outer shares LDW
                ps = psA.tile([128, 1024], f32, tag="psA")
                for fh in range(FH):
                    for sub in range(2):    # 512-col matmuls
                        sl = slice(sub * 512, (sub + 1) * 512)
                        src = slice(piece * 1024 + sub * 512,
                                    piece * 1024 + (sub + 1) * 512)
                        nc.tensor.matmul(
                            ps[:, sl], lhsT=tw_s[fh][q], rhs=xT_s[fh][:, src],
                            start=(fh == 0), stop=(fh == 1),
                        )
                nc.scalar.copy(out=m2[:, piece * 1024:(piece + 1) * 1024], in_=ps[:])
            M2.append(m2)

        if "B" in STAGES:
            stage_b(0)
            stage_b(1)

        # ---- stage C: pairwise |diff|, c-reduce, exp, j-sum --------------
        for t in range(2 if "C" in STAGES else 0):
            if "B" in STAGES and t == 1:
                stage_b(2)
                stage_b(3)                  # b-quarter pairs (q = 2t+q')
            o_sb = singles.tile([128, HW], f32, tag=f"osb{t}")
            for hwc in range(HWC):
                hsl = slice(hwc * HW_CH, (hwc + 1) * HW_CH)
                ads = []
                for qp in range(2):
                    q = 2 * t + qp
                    m2v = M2[q].rearrange("p (n hw) -> p n hw", n=N)
                    m2iv = M2i[q].rearrange("p (i hw) -> p i hw", i=NL)
                    for i in range(NL):
                        ad = adp.tile([128, N * HW_CH], f16, tag="ad")
                        adv = ad.rearrange("p (n hw) -> p n hw", n=N)
                        src0 = m2v[:, :, hsl]
                        s1 = m2iv[:, i, hsl]           # [128, 64]
                        src1 = bass.AP(
                            tensor=s1.tensor, offset=s1.offset,
                            ap=[list(s1.ap[0]), [0, N], list(s1.ap[1])],
                        )
                        if ABS_MODE == "subonly":   # timing probe: wrong numerics
                            nc.vector.tensor_sub(adv, src0, src1)
                        elif ABS_MODE == "custom":
                            bi = nc.vector._custom_dve(
                                _get_absdiff_op(), out=adv, in0=src0, in1=src1,
                            )
                            bi.ins.perf_max = PERF_MAX
                        else:
                            nc.vector.tensor_sub(adv, src0, src1)
                            nc.vector.scalar_tensor_tensor(
                                out=ad, in0=ad, scalar=-1.0, in1=ad,
                                op0=mybir.AluOpType.mult, op1=mybir.AluOpType.max,
                            )
                        ads.append(ad)
                E = Ep.tile([128, N * HW_CH], f32, tag="E")
                nrms = []
                for _h in range(4):
                    nrm = psN.tile([128, 512], f32, tag="nrm")
                    nrms.append(nrm)
                # col-tiled c-reduce: stripe s runs on col group g = s//2
                # (output partitions 32g..32g+31); the 4 col strips stream
                # concurrently, 2 accumulating stripes each.
                for sp in range(2):
                    for h in range(4):
                        for g in range(4):
                            s = 2 * g + sp
                            nc.tensor.matmul(
                                nrms[h][32 * g:32 * g + 32, :],
                                lhsT=ones_s[s],
                                rhs=ads[s][:, h * 512:(h + 1) * 512],
                                start=(sp == 0), stop=(sp == 1),
                                tile_position=(0, 32 * g),
                            )
                for h in range(4):
                    nc.scalar.activation(
                        out=E[:, h * 512:(h + 1) * 512], in_=nrms[h][:],
                        func=mybir.ActivationFunctionType.Exp, scale=-1.0,
                    )
                Ev = E.rearrange("p (j hw) -> p hw j", j=N)
                nc.vector.tensor_reduce(
                    out=o_sb[:, hsl], in_=Ev,
                    axis=mybir.AxisListType.X, op=mybir.AluOpType.add,
                )
            nc.sync.dma_start(out=o_d[t], in_=o_sb)


# --------------------------------------------------------------------------
# host side
# --------------------------------------------------------------------------

def prep_inputs(x, T):
    """Shared (core-independent) device inputs."""
    xf = np.ascontiguousarray(x.reshape(N, HW, F))
    xT_np = np.ascontiguousarray(xf.transpose(2, 0, 1).reshape(F, N * HW))
    xT_in = xT_np.reshape(FH, 128, N * HW).astype(np.float16)
    tw = T.reshape(FH, 128, Q, 16, C).transpose(0, 2, 1, 3, 4)
    tw_in = np.ascontiguousarray(tw.reshape(FH, Q, 128, 128)).astype(np.float16)
    ones_in = np.zeros((8, 128, 32), np.float16)
    for s in range(8):
        for b in range(16):
            ones_in[s, b * 8:(b + 1) * 8, 16 * (s % 2) + b] = 1.0
    return xT_np, xT_in, tw_in, ones_in


def core_in_map(xT_np, xT_in, tw_in, ones_in, k):
    xiT = np.ascontiguousarray(
        xT_np[:, k * NL * HW:(k + 1) * NL * HW]
    ).reshape(FH, 128, NL * HW).astype(np.float16)
    return {"xT": xT_in, "xiT": xiT, "tw": tw_in, "ones": ones_in}


def gather_ob(core_outs):
    """core_outs: list of 8 arrays [2,128,256] f32 -> o_b [N,16,16,B]."""
    obs = []
    for res in core_outs:
        v = res.astype(np.float32).reshape(2, 2, NL, 16, HW)   # t, q', i, b, hw
        obs.append(v.transpose(2, 4, 0, 1, 3).reshape(NL, HW, B))
    return np.concatenate(obs, axis=0).reshape(N, 16, 16, B)


_CACHED = {}


def _get_program(reps=1, loop=None):
    key = ("nc", reps, loop)
    if key in _CACHED:
        return _CACHED[key]
    from contextlib import ExitStack
    import concourse.bacc as bacc
    import concourse.mybir as mybir
    import concourse.tile as tile

    nc = bacc.Bacc("TRN2", target_bir_lowering=False, debug=False,
                   num_devices=CORES)
    f16, f32 = mybir.dt.float16, mybir.dt.float32
    ins = {
        "xT": nc.dram_tensor("xT", [FH, 128, N * HW], f16, kind="ExternalInput").ap(),
        "xiT": nc.dram_tensor("xiT", [FH, 128, NL * HW], f16, kind="ExternalInput").ap(),
        "tw": nc.dram_tensor("tw", [FH, Q, 128, 128], f16, kind="ExternalInput").ap(),
        "ones": nc.dram_tensor("ones", [8, 128, 32], f16, kind="ExternalInput").ap(),
    }
    outs = {
        "o": nc.dram_tensor("o", [2, 128, HW], f32, kind="ExternalOutput").ap(),
    }
    with tile.TileContext(nc) as tc:
        if loop:
            with ExitStack() as ctx:
                pools = make_pools(tc, ctx)
                with tc.For_i(0, loop, 1,
                              hint_engines=(mybir.EngineType.PE,
                                            mybir.EngineType.DVE)):
                    build_body(tc, outs, ins, pools=pools)
        else:
            for r in range(reps):
                build_body(tc, outs, ins, rep=r)
    nc.compile()
    _CACHED[key] = nc
    return nc


def kernel(x, T):
    x = np.asarray(x, dtype=np.float32)
    T = np.asarray(T, dtype=np.float32)
    from concourse.bass_utils import run_bass_kernel_spmd

    nc = _get_program()
    xT_np, xT_in, tw_in, ones_in = prep_inputs(x, T)
    in_maps = [core_in_map(xT_np, xT_in, tw_in, ones_in, k) for k in range(CORES)]
    res = run_bass_kernel_spmd(nc, in_maps, core_ids=list(range(CORES)))
    ob = gather_ob([r["o"] for r in res.results])
    return np.concatenate([x, ob], axis=3)



# revision 7
# speedup vs baseline: 2.2564x; 2.2298x over previous
"""MiniBatchDiscrimination Trainium2 kernel — hw-sharded symmetric version.

reference:
    M = einsum('nhwf,fbc->nhwbc', x, T)          # [N,H,W,B,C]
    norm = sum_c |M[i] - M[j]|                   # [N,N,H,W,B]
    o_b  = sum_j exp(-norm)                      # [N,H,W,B]
    out  = concat([x, o_b], axis=3)              # [N,H,W,F+B]

Sharding: the whole computation is embarrassingly parallel over the HW=256
spatial positions, so each core takes a 32-position hw slice and computes
ALL pairs for it — no replicated M compute and no cross-core traffic.

Pair symmetry: |M_i - M_j| is symmetric, so each unordered pair is computed
once via offset streams d=1..16: stream d covers pairs (i, (i+d)%32) for all
i (d=16: i<16 only).  Each stream's exp(-norm) is accumulated into BOTH row
i (direct) and row (i+d)%32 (shifted) by TensorEngine ones-matmuls into a
single PSUM accumulator; the diagonal contributes a constant +1 fused into
the final PSUM drain.

Per-core layout:
  M2[q]   [128 part=(b16,c8), 1024 free=(i32,hwl32)] f16, per b-quarter q
  ad      [128, 1024] f16  |M2[:, i+d] - M2[:, i]|   (DVE custom op, 2X)
  nrm     [128 part=(dp2,q4,b16), 1024 free=(i,hwl)] f32 PSUM — c-reduce via
          col-tiled (128x32) stripe-ones matmuls, 2 d-slots per tile
  E       exp(-nrm) f16 (ACT)
  o_ps    [64 part=(q4,b16), 1024 free=(i,hwl)] f32 PSUM — j-sum via
          col-tiled fold matmuls (direct + d-shifted reads of E)
"""

import os
import sys

for _p in ("/opt/trn_rl_repo", "/opt/pypackages"):
    if _p not in sys.path and os.path.isdir(_p):
        sys.path.append(_p)

import numpy as np

N, HWL, F, B, C = 32, 32, 256, 64, 8
HW = 256
CORES = 8
FH = 2          # f in two partition halves of 128
Q = 4           # b-quarters of 16

F16 = "float16"


def _absdiff_uop_1x():
    """REGULAR program: |a-b| via SUB, reverse-SUB, MAX on slices 0-2."""
    from concourse.dve_uop import (
        ENABLE, AluInp, AluOp, DelayInp, InpSel, OutPath, OutSel, Trigger,
        UopConfig, UopDpConfig,
    )

    u = UopConfig()
    u.enable_input(InpSel.SRC_0, 0).enable_input(InpSel.SRC_1, 1)
    u.require_inp0 = ENABLE
    u.require_inp1 = ENABLE
    u.trigger = (Trigger.SRC_TENSOR_DONE, Trigger.NONE, Trigger.NONE)
    u.enable_output(OutSel.ALU_OUT, OutPath.WR0_LO)
    dp = u.datapath_config
    # s0: alu = a - b; carry b (chain0), capture a (chain3)
    dp[0] = (UopDpConfig()
             .enable_alu(AluOp.SUBTRACT, AluInp.PREV_ALU_OUT, AluInp.PREV_DELAY_0)
             .pass_through_delay(0)
             .enable_delay_from_src(DelayInp.PREV_ALU_OUT, 3))
    # s1: alu = b - a; capture (a-b) into chain0
    dp[1] = (UopDpConfig()
             .enable_alu(AluOp.SUBTRACT, AluInp.PREV_DELAY_0, AluInp.PREV_DELAY_3)
             .enable_delay_from_src(DelayInp.PREV_ALU_OUT, 0))
    # s2: alu = max(b-a, a-b)
    dp[2] = UopDpConfig().enable_alu(
        AluOp.MAX, AluInp.PREV_ALU_OUT, AluInp.PREV_DELAY_0)
    for i in range(3, 8):
        dp[i] = UopDpConfig().pass_through_alu()
    return u


def _absdiff_uop_2x():
    """2X_1PORT program: lo on slices 0-2, hi on slices 3-5."""
    from concourse.dve_uop import (
        ENABLE, AluInp, AluOp, DelayInp, InpSel, OutPath, OutSel, Trigger,
        UopConfig, UopDpConfig,
    )

    u = UopConfig()
    u.enable_input(InpSel.SRC_0, 0).enable_input(InpSel.SRC_1, 1)
    u.enable_input(InpSel.SRC_0_HI, 2).enable_input(InpSel.SRC_1_HI, 3)
    u.require_inp0 = ENABLE
    u.require_inp1 = ENABLE
    u.trigger = (Trigger.SRC_TENSOR_DONE, Trigger.NONE, Trigger.NONE)
    u.enable_output(OutSel.DELAY_0, OutPath.WR0_LO)   # lo result rides chain0
    u.enable_output(OutSel.ALU_OUT, OutPath.WR0_HI)   # hi result on ALU lane
    dp = u.datapath_config
    # s0: alu = a_lo - b_lo; carry b_lo(c0), a_hi(c1), b_hi(c2); capture a_lo(c3)
    dp[0] = (UopDpConfig()
             .enable_alu(AluOp.SUBTRACT, AluInp.PREV_ALU_OUT, AluInp.PREV_DELAY_0)
             .pass_through_delay(0, 1, 2)
             .enable_delay_from_src(DelayInp.PREV_ALU_OUT, 3))
    # s1: alu = b_lo - a_lo; capture (a-b)_lo into c0; carry a_hi, b_hi
    dp[1] = (UopDpConfig()
             .enable_alu(AluOp.SUBTRACT, AluInp.PREV_DELAY_0, AluInp.PREV_DELAY_3)
             .enable_delay_from_src(DelayInp.PREV_ALU_OUT, 0)
             .pass_through_delay(1, 2))
    # s2: alu = max -> |a-b|_lo; carry a_hi, b_hi
    dp[2] = (UopDpConfig()
             .enable_alu(AluOp.MAX, AluInp.PREV_ALU_OUT, AluInp.PREV_DELAY_0)
             .pass_through_delay(1, 2))
    # s3: alu = a_hi - b_hi; capture lo result into c0; carry a_hi, b_hi
    dp[3] = (UopDpConfig()
             .enable_alu(AluOp.SUBTRACT, AluInp.PREV_DELAY_1, AluInp.PREV_DELAY_2)
             .enable_delay_from_src(DelayInp.PREV_ALU_OUT, 0)
             .pass_through_delay(1, 2))
    # s4: alu = b_hi - a_hi; carry lo(c0); capture (a-b)_hi into c3
    dp[4] = (UopDpConfig()
             .enable_alu(AluOp.SUBTRACT, AluInp.PREV_DELAY_2, AluInp.PREV_DELAY_1)
             .pass_through_delay(0)
             .enable_delay_from_src(DelayInp.PREV_ALU_OUT, 3))
    # s5: alu = max -> |a-b|_hi; carry lo(c0)
    dp[5] = (UopDpConfig()
             .enable_alu(AluOp.MAX, AluInp.PREV_ALU_OUT, AluInp.PREV_DELAY_3)
             .pass_through_delay(0))
    # s6, s7: pass alu (hi) + chain0 (lo)
    for i in (6, 7):
        dp[i] = UopDpConfig().pass_through_alu().pass_through_delay(0)
    return u


def _get_absdiff_op():
    """Fused |a-b| custom DVE op with a hand-written 2X_1PORT variant."""
    if "absdiff" in _CACHED:
        return _CACHED["absdiff"]
    from concourse import dve_ops
    from concourse.dve_spec import Spec, Src0, Src1, maxx
    from concourse.dve_uop import DveOpSpec

    NAME = "ABSDIFF_ANT"
    for op in dve_ops.OPS:
        if op.name == NAME:
            _CACHED["absdiff"] = op
            return op
    spec = Spec(
        body=maxx(Src0 - Src1, Src1 - Src0),
        reference=lambda in0, in1, s0, s1, imm2: np.abs(
            in0.astype(np.float32) - in1.astype(np.float32)
        ),
    )
    op = dve_ops.DveOp(NAME, spec, subdim=False, uops_sha={})
    dve_ops.OPS.append(op)
    dve_ops.CUSTOM_DVE_SPECS[op.name] = op.spec
    row = dve_ops._CUSTOM_DVE_ROW_BASE + len(dve_ops.OPS) - 1
    dve_ops._SUB_OPCODE_FOR_NAME[op.name] = row
    compiled = DveOpSpec(
        name=NAME,
        opcode=row,
        uops=[_absdiff_uop_1x()],
        uops_2x=[_absdiff_uop_2x()],
        perf_max=1,
        rd1_en=True,
    )
    compiled.validate("v3")
    dve_ops._COMPILE_CACHE[(NAME, "v3")] = compiled
    dve_ops._COMPILE_CACHE[(NAME, "v4")] = compiled
    _CACHED["absdiff"] = op
    return op


# --------------------------------------------------------------------------
# device program
# --------------------------------------------------------------------------

def make_pools(tc, ctx, rep=0):
    sfx = f"_{rep}"
    singles = ctx.enter_context(tc.tile_pool(name="singles" + sfx, bufs=1))
    psA = ctx.enter_context(tc.tile_pool(name="psA" + sfx, bufs=2, space="PSUM"))
    psN = ctx.enter_context(tc.tile_pool(name="psN" + sfx, bufs=1, space="PSUM"))
    psO = ctx.enter_context(tc.tile_pool(name="psO" + sfx, bufs=1, space="PSUM"))
    adp = ctx.enter_context(tc.tile_pool(name="adp" + sfx, bufs=18))
    Ep = ctx.enter_context(tc.tile_pool(name="Ep" + sfx, bufs=3))
    return singles, psA, psN, psO, adp, Ep


def build_body(tc, outs, ins, rep=0, pools=None):
    """Trace the per-core Tile program.

    ins:  xT   [2,128,1024] f16  xT[fh,f,i*32+hwl] = x[i, hw(core,hwl), fh*128+f]
          tw   [2,4,128,128] f16 tw[fh,q,f,b*8+c] = T[fh*128+f,16q+b,c]
          wts  [14,128,32]  f16  0-7: stripe-ones (c-reduce), 8-9: direct
                                 folds per col group, 10-13: half folds (dp,gp)
    outs: o    [64,1024]    f32  o[16q+b, i*32+hwl] = o_b[i, hw(core,hwl), 16q+b]
    """
    from contextlib import ExitStack

    import concourse.mybir as mybir

    nc = tc.nc
    f16 = mybir.dt.float16
    f32 = mybir.dt.float32

    xT_d, tw_d, wts_d = ins["xT"], ins["tw"], ins["wts"]
    o_d = outs["o"]

    with ExitStack() as ctx:
        if pools is None:
            pools = make_pools(tc, ctx, rep)
        singles, psA, psN, psO, adp, Ep = pools

        # ---- loads -------------------------------------------------------
        xT_s, tw_s = [], []
        for fh in range(FH):
            t = singles.tile([128, N * HWL], f16, tag=f"xT{fh}")
            nc.sync.dma_start(out=t, in_=xT_d[fh])
            xT_s.append(t)
            row = []
            for q in range(Q):
                t = singles.tile([128, 128], f16, tag=f"tw{fh}{q}")
                nc.sync.dma_start(out=t, in_=tw_d[fh, q])
                row.append(t)
            tw_s.append(row)
        wts_s = []
        for w in range(14):
            t = singles.tile([128, 32], f16, tag=f"wts{w}")
            nc.sync.dma_start(out=t, in_=wts_d[w])
            wts_s.append(t)
        ones_s = wts_s[0:8]
        fold_s = wts_s[8:10]            # fold_s[gp]
        hfold_s = [wts_s[10:12], wts_s[12:14]]   # hfold_s[dp][gp]

        # ---- stage B: M2[q] = (x_slice @ T_q) in (b,c)-partition layout --
        M2 = []
        for q in range(Q):
            ps = psA.tile([128, 1024], f32, tag="psA")
            for fh in range(FH):
                for sub in range(2):
                    sl = slice(sub * 512, (sub + 1) * 512)
                    nc.tensor.matmul(
                        ps[:, sl], lhsT=tw_s[fh][q], rhs=xT_s[fh][:, sl],
                        start=(fh == 0), stop=(fh == 1),
                    )
            m2 = singles.tile([128, 1024], f16, tag=f"m2{q}")
            nc.scalar.copy(out=m2, in_=ps[:])
            M2.append(m2)

        # ---- stage C: d-streams ------------------------------------------
        o_ps = psO.tile([128, 1024], f32, tag="oPs")   # rows 0-63 used
        for t in range(8):
            # absdiff for the two d-slots of this group
            ads = [None] * 8
            for dp in range(2):
                d = 2 * t + 1 + dp
                for q in range(Q):
                    s = 4 * dp + q
                    ad = adp.tile([128, 1024], f16, tag="ad")
                    m = M2[q]
                    if d < 16:
                        la = (32 - d) * 32
                        bi = nc.vector._custom_dve(
                            _get_absdiff_op(), out=ad[:, 0:la],
                            in0=m[:, d * 32:1024], in1=m[:, 0:la],
                        )
                        bi.ins.perf_max = 1
                        bi = nc.vector._custom_dve(
                            _get_absdiff_op(), out=ad[:, la:1024],
                            in0=m[:, 0:d * 32], in1=m[:, la:1024],
                        )
                        bi.ins.perf_max = 1
                    else:
                        bi = nc.vector._custom_dve(
                            _get_absdiff_op(), out=ad[:, 0:512],
                            in0=m[:, 512:1024], in1=m[:, 0:512],
                        )
                        bi.ins.perf_max = 1
                    ads[s] = ad

            # c-reduce: col-tiled stripe-ones matmuls, stripe s -> col
            # group s//2, partitions 16s+b = (dp, q, b)
            nrm = psN.tile([128, 1024], f32, tag="nrm")
            for sp in range(2):
                for h in range(2):
                    for g in range(4):
                        s = 2 * g + sp
                        hs = slice(h * 512, (h + 1) * 512)
                        nc.tensor.matmul(
                            nrm[32 * g:32 * g + 32, hs],
                            lhsT=ones_s[s], rhs=ads[s][:, hs],
                            start=(sp == 0), stop=(sp == 1),
                            tile_position=(0, 32 * g),
                        )

            # exp(-nrm) -> f16; zero the unused (i>=16) half of the d=16 slot
            E = Ep.tile([128, 1024], f16, tag="E")
            nc.scalar.activation(
                out=E, in_=nrm[:],
                func=mybir.ActivationFunctionType.Exp, scale=-1.0,
            )
            if t == 7:
                nc.vector.memset(E[64:128, 512:1024], 0.0)

            # j-sum: direct (row i) + shifted (row i+d) accumulation
            for h in range(2):
                for gp in range(2):
                    hs = slice(h * 512, (h + 1) * 512)
                    nc.tensor.matmul(
                        o_ps[32 * gp:32 * gp + 32, hs],
                        lhsT=fold_s[gp], rhs=E[:, hs],
                        start=(t == 0), stop=False,
                        tile_position=(0, 32 * gp),
                        skip_group_check=True,
                    )
            for dp in range(2):
                d = 2 * t + 1 + dp
                if d == 16:
                    segs = [(512, 1024, 0)]
                else:
                    segs = [
                        (32 * d, 512, 0),
                        (512, 1024, 512 - 32 * d),
                        (0, 32 * d, 1024 - 32 * d),
                    ]
                for si, (o0, o1, r0) in enumerate(segs):
                    ln = o1 - o0
                    last = (t == 7 and dp == 1 and si == len(segs) - 1)
                    for gp in range(2):
                        nc.tensor.matmul(
                            o_ps[32 * gp:32 * gp + 32, o0:o1],
                            lhsT=hfold_s[dp][gp], rhs=E[:, r0:r0 + ln],
                            start=False, stop=last,
                            tile_position=(0, 32 * gp),
                            skip_group_check=True,
                        )

        # ---- diagonal (+1) fused into the PSUM drain, then DMA out -------
        o_sb = singles.tile([64, 1024], f32, tag="osb")
        nc.scalar.activation(
            out=o_sb, in_=o_ps[0:64, :],
            func=mybir.ActivationFunctionType.Identity, bias=1.0, scale=1.0,
        )
        nc.sync.dma_start(out=o_d, in_=o_sb)


# --------------------------------------------------------------------------
# host side
# --------------------------------------------------------------------------

def prep_inputs(x, T):
    """Shared (core-independent) device inputs."""
    xf = np.ascontiguousarray(x.reshape(N, HW, F))
    tw = T.reshape(FH, 128, Q, 16, C).transpose(0, 2, 1, 3, 4)
    tw_in = np.ascontiguousarray(tw.reshape(FH, Q, 128, 128)).astype(np.float16)
    wts_in = np.zeros((14, 128, 32), np.float16)
    for s in range(8):
        for b in range(16):
            wts_in[s, b * 8:(b + 1) * 8, 16 * (s % 2) + b] = 1.0
    for gp in range(2):
        for dp in range(2):
            for q in (2 * gp, 2 * gp + 1):
                for b in range(16):
                    col = 16 * (q - 2 * gp) + b
                    wts_in[8 + gp, 64 * dp + 16 * q + b, col] = 1.0
                    wts_in[10 + 2 * dp + gp, 64 * dp + 16 * q + b, col] = 1.0
    return xf, tw_in, wts_in


def core_in_map(xf, tw_in, wts_in, k):
    xs = xf[:, k * HWL:(k + 1) * HWL, :]          # [i, hwl, f]
    xT = np.ascontiguousarray(xs.transpose(2, 0, 1).reshape(FH, 128, N * HWL))
    return {"xT": xT.astype(np.float16), "tw": tw_in, "wts": wts_in}


def gather_ob(core_outs):
    """core_outs: list of 8 arrays [64,1024] f32 -> o_b [N,16,16,B]."""
    obs = []
    for res in core_outs:
        v = res.astype(np.float32).reshape(B, N, HWL)   # (16q+b), i, hwl
        obs.append(v.transpose(1, 2, 0))                # i, hwl, b
    return np.concatenate(obs, axis=1).reshape(N, 16, 16, B)


_CACHED = {}


def _get_program(reps=1, loop=None):
    key = ("nc", reps, loop)
    if key in _CACHED:
        return _CACHED[key]
    from contextlib import ExitStack
    import concourse.bacc as bacc
    import concourse.mybir as mybir
    import concourse.tile as tile

    nc = bacc.Bacc("TRN2", target_bir_lowering=False, debug=False,
                   num_devices=CORES)
    f16, f32 = mybir.dt.float16, mybir.dt.float32
    ins = {
        "xT": nc.dram_tensor("xT", [FH, 128, N * HWL], f16, kind="ExternalInput").ap(),
        "tw": nc.dram_tensor("tw", [FH, Q, 128, 128], f16, kind="ExternalInput").ap(),
        "wts": nc.dram_tensor("wts", [14, 128, 32], f16, kind="ExternalInput").ap(),
    }
    outs = {
        "o": nc.dram_tensor("o", [64, N * HWL], f32, kind="ExternalOutput").ap(),
    }
    with tile.TileContext(nc) as tc:
        if loop:
            with ExitStack() as ctx:
                pools = make_pools(tc, ctx)
                with tc.For_i(0, loop, 1,
                              hint_engines=(mybir.EngineType.PE,
                                            mybir.EngineType.DVE)):
                    build_body(tc, outs, ins, pools=pools)
        else:
            for r in range(reps):
                build_body(tc, outs, ins, rep=r)
    nc.compile()
    _CACHED[key] = nc
    return nc


def kernel(x, T):
    x = np.asarray(x, dtype=np.float32)
    T = np.asarray(T, dtype=np.float32)
    from concourse.bass_utils import run_bass_kernel_spmd

    nc = _get_program()
    xf, tw_in, wts_in = prep_inputs(x, T)
    in_maps = [core_in_map(xf, tw_in, wts_in, k) for k in range(CORES)]
    res = run_bass_kernel_spmd(nc, in_maps, core_ids=list(range(CORES)))
    ob = gather_ob([r["o"] for r in res.results])
    return np.concatenate([x, ob], axis=3)


# revision 12
# speedup vs baseline: 2.4344x; 1.0789x over previous
"""MiniBatchDiscrimination Trainium2 kernel — hw-sharded symmetric version.

reference:
    M = einsum('nhwf,fbc->nhwbc', x, T)          # [N,H,W,B,C]
    norm = sum_c |M[i] - M[j]|                   # [N,N,H,W,B]
    o_b  = sum_j exp(-norm)                      # [N,H,W,B]
    out  = concat([x, o_b], axis=3)              # [N,H,W,F+B]

Sharding: the whole computation is embarrassingly parallel over the HW=256
spatial positions, so each core takes a 32-position hw slice and computes
ALL pairs for it — no replicated M compute and no cross-core traffic.

Pair symmetry: |M_i - M_j| is symmetric, so each unordered pair is computed
once via offset streams d=1..16: stream d covers pairs (i, (i+d)%32) for all
i (d=16: i<16 only).  Each stream's exp(-norm) is accumulated into BOTH row
i (direct) and row (i+d)%32 (shifted) by TensorEngine ones-matmuls into a
single PSUM accumulator; the diagonal contributes a constant +1 fused into
the final PSUM drain.

Per-core layout:
  M2[q]   [128 part=(b16,c8), 1024 free=(i32,hwl32)] f16, per b-quarter q
  ad      [128, 1024] f16  |M2[:, i+d] - M2[:, i]|   (DVE custom op, 2X)
  nrm     [128 part=(dp2,q4,b16), 1024 free=(i,hwl)] f32 PSUM — c-reduce via
          col-tiled (128x32) stripe-ones matmuls, 2 d-slots per tile
  E       exp(-nrm) f16 (ACT)
  o_ps    [64 part=(q4,b16), 1024 free=(i,hwl)] f32 PSUM — j-sum via
          col-tiled fold matmuls (direct + d-shifted reads of E)
"""

import os
import sys

for _p in ("/opt/trn_rl_repo", "/opt/pypackages"):
    if _p not in sys.path and os.path.isdir(_p):
        sys.path.append(_p)

import numpy as np

N, HWL, F, B, C = 32, 32, 256, 64, 8
HW = 256
CORES = 8
FH = 2          # f in two partition halves of 128
Q = 4           # b-quarters of 16

F16 = "float16"


def _absdiff_uop_1x():
    """REGULAR program: |a-b| via SUB, reverse-SUB, MAX on slices 0-2."""
    from concourse.dve_uop import (
        ENABLE, AluInp, AluOp, DelayInp, InpSel, OutPath, OutSel, Trigger,
        UopConfig, UopDpConfig,
    )

    u = UopConfig()
    u.enable_input(InpSel.SRC_0, 0).enable_input(InpSel.SRC_1, 1)
    u.require_inp0 = ENABLE
    u.require_inp1 = ENABLE
    u.trigger = (Trigger.SRC_TENSOR_DONE, Trigger.NONE, Trigger.NONE)
    u.enable_output(OutSel.ALU_OUT, OutPath.WR0_LO)
    dp = u.datapath_config
    # s0: alu = a - b; carry b (chain0), capture a (chain3)
    dp[0] = (UopDpConfig()
             .enable_alu(AluOp.SUBTRACT, AluInp.PREV_ALU_OUT, AluInp.PREV_DELAY_0)
             .pass_through_delay(0)
             .enable_delay_from_src(DelayInp.PREV_ALU_OUT, 3))
    # s1: alu = b - a; capture (a-b) into chain0
    dp[1] = (UopDpConfig()
             .enable_alu(AluOp.SUBTRACT, AluInp.PREV_DELAY_0, AluInp.PREV_DELAY_3)
             .enable_delay_from_src(DelayInp.PREV_ALU_OUT, 0))
    # s2: alu = max(b-a, a-b)
    dp[2] = UopDpConfig().enable_alu(
        AluOp.MAX, AluInp.PREV_ALU_OUT, AluInp.PREV_DELAY_0)
    for i in range(3, 8):
        dp[i] = UopDpConfig().pass_through_alu()
    return u


def _absdiff_uop_2x():
    """2X_1PORT program: lo on slices 0-2, hi on slices 3-5."""
    from concourse.dve_uop import (
        ENABLE, AluInp, AluOp, DelayInp, InpSel, OutPath, OutSel, Trigger,
        UopConfig, UopDpConfig,
    )

    u = UopConfig()
    u.enable_input(InpSel.SRC_0, 0).enable_input(InpSel.SRC_1, 1)
    u.enable_input(InpSel.SRC_0_HI, 2).enable_input(InpSel.SRC_1_HI, 3)
    u.require_inp0 = ENABLE
    u.require_inp1 = ENABLE
    u.trigger = (Trigger.SRC_TENSOR_DONE, Trigger.NONE, Trigger.NONE)
    u.enable_output(OutSel.DELAY_0, OutPath.WR0_LO)   # lo result rides chain0
    u.enable_output(OutSel.ALU_OUT, OutPath.WR0_HI)   # hi result on ALU lane
    dp = u.datapath_config
    # s0: alu = a_lo - b_lo; carry b_lo(c0), a_hi(c1), b_hi(c2); capture a_lo(c3)
    dp[0] = (UopDpConfig()
             .enable_alu(AluOp.SUBTRACT, AluInp.PREV_ALU_OUT, AluInp.PREV_DELAY_0)
             .pass_through_delay(0, 1, 2)
             .enable_delay_from_src(DelayInp.PREV_ALU_OUT, 3))
    # s1: alu = b_lo - a_lo; capture (a-b)_lo into c0; carry a_hi, b_hi
    dp[1] = (UopDpConfig()
             .enable_alu(AluOp.SUBTRACT, AluInp.PREV_DELAY_0, AluInp.PREV_DELAY_3)
             .enable_delay_from_src(DelayInp.PREV_ALU_OUT, 0)
             .pass_through_delay(1, 2))
    # s2: alu = max -> |a-b|_lo; carry a_hi, b_hi
    dp[2] = (UopDpConfig()
             .enable_alu(AluOp.MAX, AluInp.PREV_ALU_OUT, AluInp.PREV_DELAY_0)
             .pass_through_delay(1, 2))
    # s3: alu = a_hi - b_hi; capture lo result into c0; carry a_hi, b_hi
    dp[3] = (UopDpConfig()
             .enable_alu(AluOp.SUBTRACT, AluInp.PREV_DELAY_1, AluInp.PREV_DELAY_2)
             .enable_delay_from_src(DelayInp.PREV_ALU_OUT, 0)
             .pass_through_delay(1, 2))
    # s4: alu = b_hi - a_hi; carry lo(c0); capture (a-b)_hi into c3
    dp[4] = (UopDpConfig()
             .enable_alu(AluOp.SUBTRACT, AluInp.PREV_DELAY_2, AluInp.PREV_DELAY_1)
             .pass_through_delay(0)
             .enable_delay_from_src(DelayInp.PREV_ALU_OUT, 3))
    # s5: alu = max -> |a-b|_hi; carry lo(c0)
    dp[5] = (UopDpConfig()
             .enable_alu(AluOp.MAX, AluInp.PREV_ALU_OUT, AluInp.PREV_DELAY_3)
             .pass_through_delay(0))
    # s6, s7: pass alu (hi) + chain0 (lo)
    for i in (6, 7):
        dp[i] = UopDpConfig().pass_through_alu().pass_through_delay(0)
    return u


def _get_absdiff_op():
    """Fused |a-b| custom DVE op with a hand-written 2X_1PORT variant."""
    if "absdiff" in _CACHED:
        return _CACHED["absdiff"]
    from concourse import dve_ops
    from concourse.dve_spec import Spec, Src0, Src1, maxx
    from concourse.dve_uop import DveOpSpec

    NAME = "ABSDIFF_ANT"
    for op in dve_ops.OPS:
        if op.name == NAME:
            _CACHED["absdiff"] = op
            return op
    spec = Spec(
        body=maxx(Src0 - Src1, Src1 - Src0),
        reference=lambda in0, in1, s0, s1, imm2: np.abs(
            in0.astype(np.float32) - in1.astype(np.float32)
        ),
    )
    op = dve_ops.DveOp(NAME, spec, subdim=False, uops_sha={})
    dve_ops.OPS.append(op)
    dve_ops.CUSTOM_DVE_SPECS[op.name] = op.spec
    row = dve_ops._CUSTOM_DVE_ROW_BASE + len(dve_ops.OPS) - 1
    dve_ops._SUB_OPCODE_FOR_NAME[op.name] = row
    compiled = DveOpSpec(
        name=NAME,
        opcode=row,
        uops=[_absdiff_uop_1x()],
        uops_2x=[_absdiff_uop_2x()],
        perf_max=1,
        rd1_en=True,
    )
    compiled.validate("v3")
    dve_ops._COMPILE_CACHE[(NAME, "v3")] = compiled
    dve_ops._COMPILE_CACHE[(NAME, "v4")] = compiled
    _CACHED["absdiff"] = op
    return op


# --------------------------------------------------------------------------
# device program
# --------------------------------------------------------------------------

def make_pools(tc, ctx, rep=0):
    sfx = f"_{rep}"
    singles = ctx.enter_context(tc.tile_pool(name="singles" + sfx, bufs=1))
    dbl = ctx.enter_context(tc.tile_pool(name="dbl" + sfx, bufs=2))
    psA = ctx.enter_context(tc.tile_pool(name="psA" + sfx, bufs=1, space="PSUM"))
    psN = ctx.enter_context(tc.tile_pool(name="psN" + sfx, bufs=2, space="PSUM"))
    psO = ctx.enter_context(tc.tile_pool(name="psO" + sfx, bufs=1, space="PSUM"))
    adp = ctx.enter_context(tc.tile_pool(name="adp" + sfx, bufs=6))
    Ep = ctx.enter_context(tc.tile_pool(name="Ep" + sfx, bufs=3))
    return singles, dbl, psA, psN, psO, adp, Ep


def build_body(tc, outs, ins, rep=0, pools=None):
    """Trace the per-core Tile program.

    ins:  xT   [2,128,1024] f16  xT[fh,f,i*32+hwl] = x[i, hw(core,hwl), fh*128+f]
          tw   [2,4,128,128] f16 tw[fh,q,f,b*8+c] = T[fh*128+f,16q+b,c]
          wts  [14,128,32]  f16  0-7: stripe-ones (c-reduce), 8-9: direct
                                 folds per col group, 10-13: half folds (dp,gp)
    outs: o    [64,1024]    f32  o[16q+b, i*32+hwl] = o_b[i, hw(core,hwl), 16q+b]
    """
    from contextlib import ExitStack

    import concourse.mybir as mybir

    nc = tc.nc
    f16 = mybir.dt.float16
    f32 = mybir.dt.float32

    xT_d, tw_d, wts_d = ins["xT"], ins["tw"], ins["wts"]
    o_d = outs["o"]

    with ExitStack() as ctx:
        if pools is None:
            pools = make_pools(tc, ctx, rep)
        singles, dbl, psA, psN, psO, adp, Ep = pools

        # ---- loads (one DMA per dram tensor; host packs partition-first) -
        xT_t = dbl.tile([128, FH * N * HWL], f16, tag="xT")
        nc.sync.dma_start(out=xT_t, in_=xT_d)
        xT_s = [xT_t[:, fh * 1024:(fh + 1) * 1024] for fh in range(FH)]
        tw_t = singles.tile([128, FH * Q * 128], f16, tag="tw")
        nc.sync.dma_start(out=tw_t, in_=tw_d)
        tw_s = [[tw_t[:, (fh * Q + q) * 128:(fh * Q + q + 1) * 128]
                 for q in range(Q)] for fh in range(FH)]
        wts_t = singles.tile([128, 14 * 32], f16, tag="wts")
        nc.sync.dma_start(out=wts_t, in_=wts_d)
        wts_s = [wts_t[:, w * 32:(w + 1) * 32] for w in range(14)]
        ones_s = wts_s[0:8]
        fold_s = wts_s[8:10]            # fold_s[gp]
        hfold_s = [wts_s[10:12], wts_s[12:14]]   # hfold_s[dp][gp]

        # ---- stage B: M2 = (x_slice @ T_q), (b,c)-partition layout, with
        # 512 circularly-padded columns per quarter so every d-stream is one
        # contiguous read: M2v[p, q, k] for k in [0,1536), k>=1024 wraps.
        m2all = dbl.tile([128, Q * 1536], f16, tag="m2")
        M2v = m2all.rearrange("p (q x) -> p q x", q=Q)
        for q in range(Q):
            ps = psA.tile([128, 1024], f32, tag="psA")
            for fh in range(FH):
                for sub in range(2):
                    sl = slice(sub * 512, (sub + 1) * 512)
                    nc.tensor.matmul(
                        ps[:, sl], lhsT=tw_s[fh][q], rhs=xT_s[fh][:, sl],
                        start=(fh == 0), stop=(fh == 1),
                    )
            nc.scalar.copy(out=m2all[:, q * 1536:q * 1536 + 1024], in_=ps[:])
            nc.scalar.copy(out=m2all[:, q * 1536 + 1024:(q + 1) * 1536],
                           in_=ps[:, 0:512])

        # ---- stage C: d-streams ------------------------------------------
        o_ps = psO.tile([128, 1024], f32, tag="oPs")   # rows 0-63 used
        for t in range(8):
            # absdiff for the two d-slots of this group: one DVE instruction
            # per d across all 4 quarters (3D AP over the padded M2)
            ads2 = []
            for dp in range(2):
                d = 2 * t + 1 + dp
                ad = adp.tile([128, Q * 1024], f16, tag="ad")
                adv = ad.rearrange("p (q x) -> p q x", q=Q)
                ln = 1024 if d < 16 else 512
                bi = nc.vector._custom_dve(
                    _get_absdiff_op(), out=adv[:, :, 0:ln],
                    in0=M2v[:, :, d * 32:d * 32 + ln], in1=M2v[:, :, 0:ln],
                )
                bi.ins.perf_max = 1
                ads2.append(ad)

            # c-reduce: col-tiled stripe-ones matmuls, stripe s = 4*dp+q ->
            # col group s//2, partitions 16s+b = (dp, q, b)
            nrm = psN.tile([128, 1024], f32, tag="nrm")
            for sp in range(2):
                for h in range(2):
                    for g in range(4):
                        s = 2 * g + sp
                        dp, q = s // 4, s % 4
                        hs = slice(h * 512, (h + 1) * 512)
                        nc.tensor.matmul(
                            nrm[32 * g:32 * g + 32, hs],
                            lhsT=ones_s[s],
                            rhs=ads2[dp][:, q * 1024 + h * 512:
                                          q * 1024 + (h + 1) * 512],
                            start=(sp == 0), stop=(sp == 1),
                            tile_position=(0, 32 * g),
                        )

            # exp(-nrm) -> f16; zero the unused (i>=16) half of the d=16 slot
            E = Ep.tile([128, 1024], f16, tag="E")
            nc.scalar.activation(
                out=E, in_=nrm[:],
                func=mybir.ActivationFunctionType.Exp, scale=-1.0,
            )
            if t == 7:
                nc.vector.memset(E[64:128, 512:1024], 0.0)

            # j-sum: direct (row i) + shifted (row i+d) accumulation
            for h in range(2):
                for gp in range(2):
                    hs = slice(h * 512, (h + 1) * 512)
                    nc.tensor.matmul(
                        o_ps[32 * gp:32 * gp + 32, hs],
                        lhsT=fold_s[gp], rhs=E[:, hs],
                        start=(t == 0), stop=False,
                        tile_position=(0, 32 * gp),
                        skip_group_check=True,
                    )
            for dp in range(2):
                d = 2 * t + 1 + dp
                if d == 16:
                    segs = [(512, 1024, 0)]
                else:
                    segs = [
                        (32 * d, 512, 0),
                        (512, 1024, 512 - 32 * d),
                        (0, 32 * d, 1024 - 32 * d),
                    ]
                for si, (o0, o1, r0) in enumerate(segs):
                    ln = o1 - o0
                    last = (t == 7 and dp == 1 and si == len(segs) - 1)
                    for gp in range(2):
                        nc.tensor.matmul(
                            o_ps[32 * gp:32 * gp + 32, o0:o1],
                            lhsT=hfold_s[dp][gp], rhs=E[:, r0:r0 + ln],
                            start=False, stop=last,
                            tile_position=(0, 32 * gp),
                            skip_group_check=True,
                        )

        # ---- diagonal (+1) fused into the PSUM drain, then DMA out -------
        o_sb = singles.tile([64, 1024], f32, tag="osb")
        nc.scalar.activation(
            out=o_sb, in_=o_ps[0:64, :],
            func=mybir.ActivationFunctionType.Identity, bias=1.0, scale=1.0,
        )
        nc.sync.dma_start(out=o_d, in_=o_sb)


# --------------------------------------------------------------------------
# host side
# --------------------------------------------------------------------------

def prep_inputs(x, T):
    """Shared (core-independent) device inputs, packed partition-first."""
    xf = np.ascontiguousarray(x.reshape(N, HW, F))
    tw = T.reshape(FH, 128, Q, 16, C).transpose(0, 2, 1, 3, 4)
    tw_in = tw.reshape(FH, Q, 128, 128)
    tw_in = np.ascontiguousarray(
        tw_in.transpose(2, 0, 1, 3).reshape(128, FH * Q * 128)
    ).astype(np.float16)
    wts_in = np.zeros((14, 128, 32), np.float16)
    for s in range(8):
        for b in range(16):
            wts_in[s, b * 8:(b + 1) * 8, 16 * (s % 2) + b] = 1.0
    for gp in range(2):
        for dp in range(2):
            for q in (2 * gp, 2 * gp + 1):
                for b in range(16):
                    col = 16 * (q - 2 * gp) + b
                    wts_in[8 + gp, 64 * dp + 16 * q + b, col] = 1.0
                    wts_in[10 + 2 * dp + gp, 64 * dp + 16 * q + b, col] = 1.0
    wts_in = np.ascontiguousarray(
        wts_in.transpose(1, 0, 2).reshape(128, 14 * 32))
    return xf, tw_in, wts_in


def core_in_map(xf, tw_in, wts_in, k):
    xs = xf[:, k * HWL:(k + 1) * HWL, :]          # [i, hwl, f]
    xT = xs.transpose(2, 0, 1).reshape(FH, 128, N * HWL)
    xT = np.ascontiguousarray(xT.transpose(1, 0, 2).reshape(128, FH * N * HWL))
    return {"xT": xT.astype(np.float16), "tw": tw_in, "wts": wts_in}


def gather_ob(core_outs):
    """core_outs: list of 8 arrays [64,1024] f32 -> o_b [N,16,16,B]."""
    obs = []
    for res in core_outs:
        v = res.astype(np.float32).reshape(B, N, HWL)   # (16q+b), i, hwl
        obs.append(v.transpose(1, 2, 0))                # i, hwl, b
    return np.concatenate(obs, axis=1).reshape(N, 16, 16, B)


_CACHED = {}


def _get_program(reps=1, loop=None):
    key = ("nc", reps, loop)
    if key in _CACHED:
        return _CACHED[key]
    from contextlib import ExitStack
    import concourse.bacc as bacc
    import concourse.mybir as mybir
    import concourse.tile as tile

    nc = bacc.Bacc("TRN2", target_bir_lowering=False, debug=False,
                   num_devices=CORES)
    f16, f32 = mybir.dt.float16, mybir.dt.float32
    ins = {
        "xT": nc.dram_tensor("xT", [128, FH * N * HWL], f16,
                             kind="ExternalInput").ap(),
        "tw": nc.dram_tensor("tw", [128, FH * Q * 128], f16,
                             kind="ExternalInput").ap(),
        "wts": nc.dram_tensor("wts", [128, 14 * 32], f16,
                              kind="ExternalInput").ap(),
    }
    outs = {
        "o": nc.dram_tensor("o", [64, N * HWL], f32, kind="ExternalOutput").ap(),
    }
    with tile.TileContext(nc) as tc:
        if loop:
            with ExitStack() as ctx:
                pools = make_pools(tc, ctx)
                with tc.For_i(0, loop, 1,
                              hint_engines=(mybir.EngineType.PE,
                                            mybir.EngineType.DVE)):
                    build_body(tc, outs, ins, pools=pools)
        else:
            for r in range(reps):
                build_body(tc, outs, ins, rep=r)
    nc.compile()
    _CACHED[key] = nc
    return nc


def kernel(x, T):
    x = np.asarray(x, dtype=np.float32)
    T = np.asarray(T, dtype=np.float32)
    from concourse.bass_utils import run_bass_kernel_spmd

    nc = _get_program()
    xf, tw_in, wts_in = prep_inputs(x, T)
    in_maps = [core_in_map(xf, tw_in, wts_in, k) for k in range(CORES)]
    res = run_bass_kernel_spmd(nc, in_maps, core_ids=list(range(CORES)))
    ob = gather_ob([r["o"] for r in res.results])
    return np.concatenate([x, ob], axis=3)


# revision 18
# speedup vs baseline: 2.7634x; 1.1351x over previous
"""MiniBatchDiscrimination Trainium2 kernel — hw-sharded symmetric version.

reference:
    M = einsum('nhwf,fbc->nhwbc', x, T)          # [N,H,W,B,C]
    norm = sum_c |M[i] - M[j]|                   # [N,N,H,W,B]
    o_b  = sum_j exp(-norm)                      # [N,H,W,B]
    out  = concat([x, o_b], axis=3)              # [N,H,W,F+B]

Sharding: the whole computation is embarrassingly parallel over the HW=256
spatial positions, so each core takes a 32-position hw slice and computes
ALL pairs for it — no replicated M compute and no cross-core traffic.

Pair symmetry: |M_i - M_j| is symmetric, so each unordered pair is computed
once via offset streams d=1..16: stream d covers pairs (i, (i+d)%32) for all
i (d=16: i<16 only).  Each stream's exp(-norm) is accumulated into BOTH row
i (direct) and row (i+d)%32 (shifted) by TensorEngine ones-matmuls into a
single PSUM accumulator; the diagonal contributes a constant +1 fused into
the final PSUM drain.

Per-core layout:
  M2[q]   [128 part=(b16,c8), 1024 free=(i32,hwl32)] f16, per b-quarter q
  ad      [128, 1024] f16  |M2[:, i+d] - M2[:, i]|   (DVE custom op, 2X)
  nrm     [128 part=(dp2,q4,b16), 1024 free=(i,hwl)] f32 PSUM — c-reduce via
          col-tiled (128x32) stripe-ones matmuls, 2 d-slots per tile
  E       exp(-nrm) f16 (ACT)
  o_ps    [64 part=(q4,b16), 1024 free=(i,hwl)] f32 PSUM — j-sum via
          col-tiled fold matmuls (direct + d-shifted reads of E)
"""

import os
import sys

for _p in ("/opt/trn_rl_repo", "/opt/pypackages"):
    if _p not in sys.path and os.path.isdir(_p):
        sys.path.append(_p)

import numpy as np

N, HWL, F, B, C = 32, 32, 256, 64, 8
HW = 256
CORES = 8
FH = 2          # f in two partition halves of 128
Q = 4           # b-quarters of 16

F16 = "float16"


def _absdiff_uop_1x():
    """REGULAR program: |a-b| via SUB, reverse-SUB, MAX on slices 0-2."""
    from concourse.dve_uop import (
        ENABLE, AluInp, AluOp, DelayInp, InpSel, OutPath, OutSel, Trigger,
        UopConfig, UopDpConfig,
    )

    u = UopConfig()
    u.enable_input(InpSel.SRC_0, 0).enable_input(InpSel.SRC_1, 1)
    u.require_inp0 = ENABLE
    u.require_inp1 = ENABLE
    u.trigger = (Trigger.SRC_TENSOR_DONE, Trigger.NONE, Trigger.NONE)
    u.enable_output(OutSel.ALU_OUT, OutPath.WR0_LO)
    dp = u.datapath_config
    # s0: alu = a - b; carry b (chain0), capture a (chain3)
    dp[0] = (UopDpConfig()
             .enable_alu(AluOp.SUBTRACT, AluInp.PREV_ALU_OUT, AluInp.PREV_DELAY_0)
             .pass_through_delay(0)
             .enable_delay_from_src(DelayInp.PREV_ALU_OUT, 3))
    # s1: alu = b - a; capture (a-b) into chain0
    dp[1] = (UopDpConfig()
             .enable_alu(AluOp.SUBTRACT, AluInp.PREV_DELAY_0, AluInp.PREV_DELAY_3)
             .enable_delay_from_src(DelayInp.PREV_ALU_OUT, 0))
    # s2: alu = max(b-a, a-b)
    dp[2] = UopDpConfig().enable_alu(
        AluOp.MAX, AluInp.PREV_ALU_OUT, AluInp.PREV_DELAY_0)
    for i in range(3, 8):
        dp[i] = UopDpConfig().pass_through_alu()
    return u


def _absdiff_uop_2x():
    """2X_1PORT program: lo on slices 0-2, hi on slices 3-5."""
    from concourse.dve_uop import (
        ENABLE, AluInp, AluOp, DelayInp, InpSel, OutPath, OutSel, Trigger,
        UopConfig, UopDpConfig,
    )

    u = UopConfig()
    u.enable_input(InpSel.SRC_0, 0).enable_input(InpSel.SRC_1, 1)
    u.enable_input(InpSel.SRC_0_HI, 2).enable_input(InpSel.SRC_1_HI, 3)
    u.require_inp0 = ENABLE
    u.require_inp1 = ENABLE
    u.trigger = (Trigger.SRC_TENSOR_DONE, Trigger.NONE, Trigger.NONE)
    u.enable_output(OutSel.DELAY_0, OutPath.WR0_LO)   # lo result rides chain0
    u.enable_output(OutSel.ALU_OUT, OutPath.WR0_HI)   # hi result on ALU lane
    dp = u.datapath_config
    # s0: alu = a_lo - b_lo; carry b_lo(c0), a_hi(c1), b_hi(c2); capture a_lo(c3)
    dp[0] = (UopDpConfig()
             .enable_alu(AluOp.SUBTRACT, AluInp.PREV_ALU_OUT, AluInp.PREV_DELAY_0)
             .pass_through_delay(0, 1, 2)
             .enable_delay_from_src(DelayInp.PREV_ALU_OUT, 3))
    # s1: alu = b_lo - a_lo; capture (a-b)_lo into c0; carry a_hi, b_hi
    dp[1] = (UopDpConfig()
             .enable_alu(AluOp.SUBTRACT, AluInp.PREV_DELAY_0, AluInp.PREV_DELAY_3)
             .enable_delay_from_src(DelayInp.PREV_ALU_OUT, 0)
             .pass_through_delay(1, 2))
    # s2: alu = max -> |a-b|_lo; carry a_hi, b_hi
    dp[2] = (UopDpConfig()
             .enable_alu(AluOp.MAX, AluInp.PREV_ALU_OUT, AluInp.PREV_DELAY_0)
             .pass_through_delay(1, 2))
    # s3: alu = a_hi - b_hi; capture lo result into c0; carry a_hi, b_hi
    dp[3] = (UopDpConfig()
             .enable_alu(AluOp.SUBTRACT, AluInp.PREV_DELAY_1, AluInp.PREV_DELAY_2)
             .enable_delay_from_src(DelayInp.PREV_ALU_OUT, 0)
             .pass_through_delay(1, 2))
    # s4: alu = b_hi - a_hi; carry lo(c0); capture (a-b)_hi into c3
    dp[4] = (UopDpConfig()
             .enable_alu(AluOp.SUBTRACT, AluInp.PREV_DELAY_2, AluInp.PREV_DELAY_1)
             .pass_through_delay(0)
             .enable_delay_from_src(DelayInp.PREV_ALU_OUT, 3))
    # s5: alu = max -> |a-b|_hi; carry lo(c0)
    dp[5] = (UopDpConfig()
             .enable_alu(AluOp.MAX, AluInp.PREV_ALU_OUT, AluInp.PREV_DELAY_3)
             .pass_through_delay(0))
    # s6, s7: pass alu (hi) + chain0 (lo)
    for i in (6, 7):
        dp[i] = UopDpConfig().pass_through_alu().pass_through_delay(0)
    return u


def _get_absdiff_op():
    """Fused |a-b| custom DVE op with a hand-written 2X_1PORT variant."""
    if "absdiff" in _CACHED:
        return _CACHED["absdiff"]
    from concourse import dve_ops
    from concourse.dve_spec import Spec, Src0, Src1, maxx
    from concourse.dve_uop import DveOpSpec

    NAME = "ABSDIFF_ANT"
    for op in dve_ops.OPS:
        if op.name == NAME:
            _CACHED["absdiff"] = op
            return op
    spec = Spec(
        body=maxx(Src0 - Src1, Src1 - Src0),
        reference=lambda in0, in1, s0, s1, imm2: np.abs(
            in0.astype(np.float32) - in1.astype(np.float32)
        ),
    )
    op = dve_ops.DveOp(NAME, spec, subdim=False, uops_sha={})
    dve_ops.OPS.append(op)
    dve_ops.CUSTOM_DVE_SPECS[op.name] = op.spec
    row = dve_ops._CUSTOM_DVE_ROW_BASE + len(dve_ops.OPS) - 1
    dve_ops._SUB_OPCODE_FOR_NAME[op.name] = row
    compiled = DveOpSpec(
        name=NAME,
        opcode=row,
        uops=[_absdiff_uop_1x()],
        uops_2x=[_absdiff_uop_2x()],
        perf_max=1,
        rd1_en=True,
    )
    compiled.validate("v3")
    dve_ops._COMPILE_CACHE[(NAME, "v3")] = compiled
    dve_ops._COMPILE_CACHE[(NAME, "v4")] = compiled
    _CACHED["absdiff"] = op
    return op


# --------------------------------------------------------------------------
# device program
# --------------------------------------------------------------------------

def make_pools(tc, ctx, rep=0):
    sfx = f"_{rep}"
    singles = ctx.enter_context(tc.tile_pool(name="singles" + sfx, bufs=1))
    dbl = ctx.enter_context(tc.tile_pool(name="dbl" + sfx, bufs=2))
    psA = ctx.enter_context(tc.tile_pool(name="psA" + sfx, bufs=2, space="PSUM"))
    psN = ctx.enter_context(tc.tile_pool(name="psN" + sfx, bufs=1, space="PSUM"))
    psO = ctx.enter_context(tc.tile_pool(name="psO" + sfx, bufs=1, space="PSUM"))
    adp = ctx.enter_context(tc.tile_pool(name="adp" + sfx, bufs=6))
    Ep = ctx.enter_context(tc.tile_pool(name="Ep" + sfx, bufs=3))
    return singles, dbl, psA, psN, psO, adp, Ep


def build_body(tc, outs, ins, rep=0, pools=None):
    """Trace the per-core Tile program.

    ins:  xT   [2,128,1024] f16  xT[fh,f,i*32+hwl] = x[i, hw(core,hwl), fh*128+f]
          tw   [2,4,128,128] f16 tw[fh,q,f,b*8+c] = T[fh*128+f,16q+b,c]
          wts  [14,128,32]  f16  0-7: stripe-ones (c-reduce), 8-9: direct
                                 folds per col group, 10-13: half folds (dp,gp)
    outs: o    [64,1024]    f32  o[16q+b, i*32+hwl] = o_b[i, hw(core,hwl), 16q+b]
    """
    from contextlib import ExitStack

    import concourse.mybir as mybir

    nc = tc.nc
    f16 = mybir.dt.float16
    f32 = mybir.dt.float32

    xT_d, tw_d, wts_d = ins["xT"], ins["tw"], ins["wts"]
    o_d = outs["o"]

    with ExitStack() as ctx:
        if pools is None:
            pools = make_pools(tc, ctx, rep)
        singles, dbl, psA, psN, psO, adp, Ep = pools

        # ---- loads (one DMA per dram tensor; host packs partition-first) -
        xT_t = dbl.tile([128, FH * N * HWL], f16, tag="xT")
        nc.sync.dma_start(out=xT_t, in_=xT_d)
        xT_s = [xT_t[:, fh * 1024:(fh + 1) * 1024] for fh in range(FH)]
        tw_t = singles.tile([128, FH * Q * 128], f16, tag="tw")
        nc.sync.dma_start(out=tw_t, in_=tw_d)
        tw_s = [[tw_t[:, (fh * Q + q) * 128:(fh * Q + q + 1) * 128]
                 for q in range(Q)] for fh in range(FH)]
        wts_t = singles.tile([128, 14 * 32], f16, tag="wts")
        nc.sync.dma_start(out=wts_t, in_=wts_d)
        wts_s = [wts_t[:, w * 32:(w + 1) * 32] for w in range(14)]
        ones_s = wts_s[0:8]
        fold_s = wts_s[8:10]            # fold_s[gp]
        hfold_s = [wts_s[10:12], wts_s[12:14]]   # hfold_s[dp][gp]

        # ---- stage B: M2 = (x_slice @ T_q), (b,c)-partition layout, with
        # 512 circularly-padded columns per quarter so every d-stream is one
        # contiguous read: M2v[p, q, k] for k in [0,1536), k>=1024 wraps.
        m2all = dbl.tile([128, Q * 1536], f16, tag="m2")
        M2v = m2all.rearrange("p (q x) -> p q x", q=Q)
        for q in range(Q):
            ps = psA.tile([128, 1024], f32, tag="psA")
            for fh in range(FH):
                for sub in range(2):
                    sl = slice(sub * 512, (sub + 1) * 512)
                    nc.tensor.matmul(
                        ps[:, sl], lhsT=tw_s[fh][q], rhs=xT_s[fh][:, sl],
                        start=(fh == 0), stop=(fh == 1),
                    )
            nc.scalar.copy(out=m2all[:, q * 1536:q * 1536 + 1024], in_=ps[:])
            nc.scalar.copy(out=m2all[:, q * 1536 + 1024:(q + 1) * 1536],
                           in_=ps[:, 0:512])

        # ---- stage C: d-streams ------------------------------------------
        o_ps = psO.tile([128, 1024], f32, tag="oPs")   # rows 0-63 used
        for t in range(8):
            # absdiff for the two d-slots of this group.  Early groups go
            # per-quarter so the DVE starts as soon as stage B's first
            # quarter lands; later groups use one 3D-AP instruction per d.
            ads2 = []
            if t < 2:
                ad0 = adp.tile([128, Q * 1024], f16, tag="ad")
                ad1 = adp.tile([128, Q * 1024], f16, tag="ad")
                ads2 = [ad0, ad1]
                for q in range(Q):
                    for dp in range(2):
                        d = 2 * t + 1 + dp
                        bi = nc.vector._custom_dve(
                            _get_absdiff_op(),
                            out=ads2[dp][:, q * 1024:(q + 1) * 1024],
                            in0=M2v[:, q, d * 32:d * 32 + 1024],
                            in1=M2v[:, q, 0:1024],
                        )
                        bi.ins.perf_max = 1
            else:
                for dp in range(2):
                    d = 2 * t + 1 + dp
                    ad = adp.tile([128, Q * 1024], f16, tag="ad")
                    adv = ad.rearrange("p (q x) -> p q x", q=Q)
                    ln = 1024 if d < 16 else 512
                    bi = nc.vector._custom_dve(
                        _get_absdiff_op(), out=adv[:, :, 0:ln],
                        in0=M2v[:, :, d * 32:d * 32 + ln], in1=M2v[:, :, 0:ln],
                    )
                    bi.ins.perf_max = 1
                    ads2.append(ad)

            # c-reduce: col-tiled stripe-ones matmuls, stripe s = 4*dp+q ->
            # col group s//2, partitions 16s+b = (dp, q, b)
            nrm = psN.tile([128, 1024], f32, tag="nrm")
            for sp in range(2):
                for h in range(2):
                    for g in range(4):
                        s = 2 * g + sp
                        dp, q = s // 4, s % 4
                        hs = slice(h * 512, (h + 1) * 512)
                        nc.tensor.matmul(
                            nrm[32 * g:32 * g + 32, hs],
                            lhsT=ones_s[s],
                            rhs=ads2[dp][:, q * 1024 + h * 512:
                                          q * 1024 + (h + 1) * 512],
                            start=(sp == 0), stop=(sp == 1),
                            tile_position=(0, 32 * g),
                        )

            # exp(-nrm) -> f16; the last group splits by half and leaves the
            # unused (i>=16) region of the d=16 slot to an early memset so
            # nothing serializes the tail.
            E = Ep.tile([128, 1024], f16, tag="E")
            if t == 7:
                nc.vector.memset(E[64:128, 512:1024], 0.0)
                nc.scalar.activation(
                    out=E[:, 0:512], in_=nrm[:, 0:512],
                    func=mybir.ActivationFunctionType.Exp, scale=-1.0,
                )
                nc.scalar.activation(
                    out=E[0:64, 512:1024], in_=nrm[0:64, 512:1024],
                    func=mybir.ActivationFunctionType.Exp, scale=-1.0,
                )
            else:
                nc.scalar.activation(
                    out=E, in_=nrm[:],
                    func=mybir.ActivationFunctionType.Exp, scale=-1.0,
                )

            # j-sum: direct (row i) + shifted (row i+d) accumulation
            for h in range(2):
                for gp in range(2):
                    hs = slice(h * 512, (h + 1) * 512)
                    nc.tensor.matmul(
                        o_ps[32 * gp:32 * gp + 32, hs],
                        lhsT=fold_s[gp], rhs=E[:, hs],
                        start=(t == 0), stop=False,
                        tile_position=(0, 32 * gp),
                        skip_group_check=True,
                    )
            for dp in range(2):
                d = 2 * t + 1 + dp
                if d == 16:
                    segs = [(512, 1024, 0)]
                else:
                    segs = [
                        (32 * d, 512, 0),
                        (512, 1024, 512 - 32 * d),
                        (0, 32 * d, 1024 - 32 * d),
                    ]
                for si, (o0, o1, r0) in enumerate(segs):
                    ln = o1 - o0
                    last = (t == 7 and dp == 1 and si == len(segs) - 1)
                    for gp in range(2):
                        nc.tensor.matmul(
                            o_ps[32 * gp:32 * gp + 32, o0:o1],
                            lhsT=hfold_s[dp][gp], rhs=E[:, r0:r0 + ln],
                            start=False, stop=last,
                            tile_position=(0, 32 * gp),
                            skip_group_check=True,
                        )

        # ---- diagonal (+1) fused into the PSUM drain, then DMA out -------
        o_sb = singles.tile([64, 1024], f32, tag="osb")
        nc.scalar.activation(
            out=o_sb, in_=o_ps[0:64, :],
            func=mybir.ActivationFunctionType.Identity, bias=1.0, scale=1.0,
        )
        nc.sync.dma_start(out=o_d, in_=o_sb)


# --------------------------------------------------------------------------
# host side
# --------------------------------------------------------------------------

def prep_inputs(x, T):
    """Shared (core-independent) device inputs, packed partition-first."""
    xf = np.ascontiguousarray(x.reshape(N, HW, F))
    tw = T.reshape(FH, 128, Q, 16, C).transpose(0, 2, 1, 3, 4)
    tw_in = tw.reshape(FH, Q, 128, 128)
    tw_in = np.ascontiguousarray(
        tw_in.transpose(2, 0, 1, 3).reshape(128, FH * Q * 128)
    ).astype(np.float16)
    wts_in = np.zeros((14, 128, 32), np.float16)
    for s in range(8):
        for b in range(16):
            wts_in[s, b * 8:(b + 1) * 8, 16 * (s % 2) + b] = 1.0
    for gp in range(2):
        for dp in range(2):
            for q in (2 * gp, 2 * gp + 1):
                for b in range(16):
                    col = 16 * (q - 2 * gp) + b
                    wts_in[8 + gp, 64 * dp + 16 * q + b, col] = 1.0
                    wts_in[10 + 2 * dp + gp, 64 * dp + 16 * q + b, col] = 1.0
    wts_in = np.ascontiguousarray(
        wts_in.transpose(1, 0, 2).reshape(128, 14 * 32))
    return xf, tw_in, wts_in


def core_in_map(xf, tw_in, wts_in, k):
    xs = xf[:, k * HWL:(k + 1) * HWL, :]          # [i, hwl, f]
    xT = xs.transpose(2, 0, 1).reshape(FH, 128, N * HWL)
    xT = np.ascontiguousarray(xT.transpose(1, 0, 2).reshape(128, FH * N * HWL))
    return {"xT": xT.astype(np.float16), "tw": tw_in, "wts": wts_in}


def gather_ob(core_outs):
    """core_outs: list of 8 arrays [64,1024] f32 -> o_b [N,16,16,B]."""
    obs = []
    for res in core_outs:
        v = res.astype(np.float32).reshape(B, N, HWL)   # (16q+b), i, hwl
        obs.append(v.transpose(1, 2, 0))                # i, hwl, b
    return np.concatenate(obs, axis=1).reshape(N, 16, 16, B)


_CACHED = {}


def _get_program(reps=1, loop=None):
    key = ("nc", reps, loop)
    if key in _CACHED:
        return _CACHED[key]
    from contextlib import ExitStack
    import concourse.bacc as bacc
    import concourse.mybir as mybir
    import concourse.tile as tile

    nc = bacc.Bacc("TRN2", target_bir_lowering=False, debug=False,
                   num_devices=CORES)
    f16, f32 = mybir.dt.float16, mybir.dt.float32
    ins = {
        "xT": nc.dram_tensor("xT", [128, FH * N * HWL], f16,
                             kind="ExternalInput").ap(),
        "tw": nc.dram_tensor("tw", [128, FH * Q * 128], f16,
                             kind="ExternalInput").ap(),
        "wts": nc.dram_tensor("wts", [128, 14 * 32], f16,
                              kind="ExternalInput").ap(),
    }
    outs = {
        "o": nc.dram_tensor("o", [64, N * HWL], f32, kind="ExternalOutput").ap(),
    }
    with tile.TileContext(nc) as tc:
        if loop:
            with ExitStack() as ctx:
                pools = make_pools(tc, ctx)
                with tc.For_i(0, loop, 1,
                              hint_engines=(mybir.EngineType.PE,
                                            mybir.EngineType.DVE)):
                    build_body(tc, outs, ins, pools=pools)
        else:
            for r in range(reps):
                build_body(tc, outs, ins, rep=r)
    nc.compile()
    _CACHED[key] = nc
    return nc


def kernel(x, T):
    x = np.asarray(x, dtype=np.float32)
    T = np.asarray(T, dtype=np.float32)
    from concourse.bass_utils import run_bass_kernel_spmd

    nc = _get_program()
    xf, tw_in, wts_in = prep_inputs(x, T)
    in_maps = [core_in_map(xf, tw_in, wts_in, k) for k in range(CORES)]
    res = run_bass_kernel_spmd(nc, in_maps, core_ids=list(range(CORES)))
    ob = gather_ob([r["o"] for r in res.results])
    return np.concatenate([x, ob], axis=3)
